# revision 18
# baseline (speedup 1.0000x reference)
"""Host-side sharding/prep + Bass device program for nn_BBGRUDecoder.

Host code does index manipulation / data layout only (no model FLOPs);
the device kernel does all arithmetic.

Device-side structure (per core, SPMD over 8 cores; shots sharded
round-robin by shot % 8):

  conv1: node features staged transposed as [85, U_pad] bf16 where rows
    0..79 are the 16 in-edge slots x 5 features and rows 80..84 are the
    node's own features (eav rows are edge weights, 1.0 for the root
    block). msg = xs*eav elementwise, then ONE matmul per tile against
    w1full [85,128] (wrel tiled 16x + wroot) folds the slot reduction,
    the root term and the F1 projection. V tiles produce hresT [F1,node]
    (SBUF-resident, feeds conv2's root term); SV + halo tiles produce
    [node,F1] rows written to the DRAM gather table tabA.
  conv2: per 128-edge subtile, gather source rows from tabA (Pool SWDGE)
    and matmul against the sparse scatter matrix s2 (flat [128,E2_slots]
    layout for full-rate slab loads). Root term from hresT. Pool by
    graph via s3 0/1 matrices, scatter per-graph sums to emb (indirect
    DMA, graph-column ids).
  GRU: emb -> embT [128, GCOLS], batched input projections, 10-step
    recurrence x 2 layers, decoder matmul, active-shot mask.
"""
import numpy as np
import ml_dtypes

BF16 = np.dtype(ml_dtypes.bfloat16)
NC = 8
P = 128
KSLOT = 16       # conv1 in-edge slots per node (max in-degree 13)
F_IN = 5
F1 = 128
F2 = 256
HID = 128
TR = 10          # rounds per shot
MAXG_TILE = 32   # max graphs per node-tile (pool S3 width)
IDX_CAP = 32767
XROWS = KSLOT * F_IN + F_IN   # 85: root rows (first 5) + slot rows
GRP = 8          # node tiles per conv1 slab
GG = 16          # conv2 edge subtiles per gather group
WW = 32          # conv2 dst-window width (s2 block cols)
NWIN = P // WW   # dst windows per node tile


def _pack_groups(sizes, cap_items, cap_groups):
    """Greedy-pack consecutive groups (each <=cap_items items) into tiles of
    <=cap_items items and <=cap_groups groups. Returns list of (start_group,
    n_groups, n_items)."""
    tiles = []
    i = 0
    n = len(sizes)
    while i < n:
        items = 0
        g = 0
        while i + g < n and g < cap_groups and items + sizes[i + g] <= cap_items:
            items += sizes[i + g]
            g += 1
        assert g > 0, f"group {i} size {sizes[i]} exceeds cap {cap_items}"
        tiles.append((i, g, items))
        i += g
    return tiles


def prep(inputs):
    x = np.asarray(inputs["x"], np.float32)
    ei = np.asarray(inputs["edge_index"], np.int64)
    ea = np.asarray(inputs["edge_attr"], np.float32)
    bl = np.asarray(inputs["batch_labels"], np.int64)
    lm = np.asarray(inputs["label_map"], np.int64)
    B = int(inputs["B"])
    NN = x.shape[0]
    src_g, dst_g = ei[0], ei[1]
    shot_of, round_of = lm[:, 0], lm[:, 1]
    n_shot_core = (B + NC - 1) // NC          # 128 shots per core
    GCOLS = n_shot_core * TR                  # 1280 graph-columns per core
    deg = np.bincount(dst_g, minlength=NN)
    assert deg.max() <= KSLOT

    node_g = bl
    node_core = (shot_of[node_g] % NC).astype(np.int64)

    cores = []
    for d in range(NC):
        V = np.nonzero(node_core == d)[0]          # ascending node ids
        gids, gstart, gcnt = np.unique(node_g[V], return_index=True, return_counts=True)
        # order graphs by max node in-degree so slab XR (=5+5*maxdeg) shrinks
        gmax = np.maximum.reduceat(deg[V], gstart)
        gord = np.argsort(gmax, kind="stable")
        gids, gstart, gcnt = gids[gord], gstart[gord], gcnt[gord]
        # graph column within core: s_idx*TR + round
        s_idx = shot_of[gids] // NC
        # round-major: emb tile t holds exactly the round-t embeddings,
        # so the GRU consumes emb tiles in step order (pipelines with loads)
        gcol = round_of[gids] * n_shot_core + s_idx
        # ---- graph-aligned node tiles ----
        tiles = _pack_groups(gcnt.tolist(), P, MAXG_TILE)
        NT_V = len(tiles)
        vpos = np.full(NN, -1, np.int64)        # global node id -> packed pos
        packed_rows = []                        # per tile: global node ids (len<=P)
        tile_graphs = []                        # per tile: (gcol list, counts)
        for (g0, ng, ni) in tiles:
            rows = [V[gstart[k]:gstart[k] + gcnt[k]] for k in range(g0, g0 + ng)]
            rows = np.concatenate(rows)
            packed_rows.append(rows)
            tile_graphs.append((gcol[g0:g0 + ng], gcnt[g0:g0 + ng]))
        for t, rows in enumerate(packed_rows):
            vpos[rows] = t * P + np.arange(len(rows))

        # ---- conv2 edges (dst in V) ----
        E = np.nonzero(node_core[dst_g] == d)[0]
        e_src, e_dst, e_ea = src_g[E], dst_g[E], ea[E]
        S = np.unique(e_src)
        SV = S[node_core[S] == d]
        H = S[node_core[S] != d]
        H = H[np.argsort(deg[H], kind="stable")]   # degree-sorted slabs
        # table A (gather source) layout: [SV (degree-sorted), H]
        SVo = SV[np.argsort(deg[SV], kind="stable")]
        apos = np.full(NN, -1, np.int64)
        apos[SVo] = np.arange(len(SVo))
        cores.append(dict(
            d=d, V=V, NT_V=NT_V, tiles=tiles, packed_rows=packed_rows,
            tile_graphs=tile_graphs, vpos=vpos,
            E=E, e_src=e_src, e_dst=e_dst, e_ea=e_ea,
            SVo=SVo, H=H, apos=apos, gids=gids, gcol=gcol, gcnt=gcnt,
        ))

    # ---- shared static shapes ----
    NT_V = max(c["NT_V"] for c in cores)
    NT_V = -(-NT_V // GRP) * GRP              # pad to slab multiple
    n_SV_pad = -(-max(len(c["SVo"]) for c in cores) // (GRP * P)) * (GRP * P)
    NT_H = -(-max(len(c["H"]) for c in cores) // P)
    NT_H = -(-NT_H // GRP) * GRP
    H_pad = NT_H * P
    A_rows = n_SV_pad + H_pad
    assert A_rows <= IDX_CAP + 1, A_rows
    V_pad = NT_V * P

    # conv2: per-tile x per-dst-window edge subtile counts (max over cores).
    # Each window covers WW consecutive dst slots; a subtile is <=128 edges
    # whose dsts fall in that window -> s2 block is [128, WW] not [128, 128].
    wsub = np.ones((NT_V, NWIN), np.int64)
    for c in cores:
        vp = c["vpos"][c["e_dst"]]
        cnt = np.bincount(vp // WW, minlength=NT_V * NWIN)[:NT_V * NWIN]
        ns = np.maximum(-(-cnt // P), 1).reshape(-1, NWIN)
        n_t = ns.shape[0]
        wsub[:n_t] = np.maximum(wsub[:n_t], ns)
    T_sub = wsub.sum(axis=1)
    n_slots = int(T_sub.sum())
    E2_slots = n_slots * P

    # per-slab conv1 staging rows: XR = 5 (root) + 5*max_indegree_in_slab
    NSL_V_ = NT_V // GRP
    NSL_SV_ = n_SV_pad // (GRP * P)
    NSL_H_ = NT_H // GRP
    KV = np.zeros(NSL_V_, np.int64)
    KSV = np.zeros(NSL_SV_, np.int64)
    KH = np.zeros(NSL_H_, np.int64)
    for c in cores:
        for s in range(NSL_V_):
            rows = [c["packed_rows"][t]
                    for t in range(s * GRP, min((s + 1) * GRP, c["NT_V"]))]
            rows = np.concatenate(rows) if rows else np.zeros(0, np.int64)
            if len(rows):
                KV[s] = max(KV[s], deg[rows].max())
        for s in range(NSL_SV_):
            rows = c["SVo"][s * GRP * P:(s + 1) * GRP * P]
            if len(rows):
                KSV[s] = max(KSV[s], deg[rows].max())
        for s in range(NSL_H_):
            rows = c["H"][s * GRP * P:(s + 1) * GRP * P]
            if len(rows):
                KH[s] = max(KH[s], deg[rows].max())
    XR_V = (F_IN + F_IN * KV).tolist()
    XR_SV = (F_IN + F_IN * KSV).tolist()
    XR_H = (F_IN + F_IN * KH).tolist()

    meta = dict(NT_V=NT_V, NT_H=NT_H, V_pad=V_pad, H_pad=H_pad,
                n_SV_pad=n_SV_pad, A_rows=A_rows, T_sub=T_sub.tolist(),
                wsub=wsub.tolist(), XR_V=XR_V, XR_SV=XR_SV, XR_H=XR_H,
                E2_slots=E2_slots, GCOLS=GCOLS, G_rows=-(-(GCOLS + 1) // P) * P,
                n_shot_core=n_shot_core, B=B)

    # ---- per-core padded arrays ----
    ones5 = np.ones(F_IN, np.float32)
    for c in cores:
        vpos, apos = c["vpos"], c["apos"]

        def slot_arrays(nodes, npad):
            """Transposed conv1 staging [XROWS, 2, npad] for the given
            node list (bf16): [:,0]=feature values, [:,1]=edge weights."""
            xs = np.zeros((npad, KSLOT, F_IN), np.float32)
            ev = np.zeros((npad, KSLOT, F_IN), np.float32)
            pos = np.full(NN, -1, np.int64)
            pos[nodes] = np.arange(len(nodes))
            # in-edges of these nodes from FULL edge list
            EU = np.nonzero(pos[dst_g] >= 0)[0]
            du = pos[dst_g[EU]]
            order = np.argsort(du, kind="stable")
            EU, du = EU[order], du[order]
            # vectorized running slot index within each dst group
            uniq, first = np.unique(du, return_index=True)
            sl = np.arange(len(EU)) - np.repeat(first, np.diff(
                np.concatenate([first, [len(EU)]])))
            xs[du, sl] = x[src_g[EU]]
            ev[du, sl] = ea[EU][:, None] * ones5
            out = np.zeros((XROWS, 2, npad), np.float32)
            out[:F_IN, 0, :len(nodes)] = x[nodes].T
            out[:F_IN, 1, :len(nodes)] = 1.0
            out[F_IN:, 0] = xs.reshape(npad, -1).T
            out[F_IN:, 1] = ev.reshape(npad, -1).T
            return out.astype(BF16)

        U_nodes = np.full(V_pad, -1, np.int64)
        for t, rows in enumerate(c["packed_rows"]):
            U_nodes[t * P:t * P + len(rows)] = rows
        # U_nodes has -1 gaps between tiles; compute compact, scatter to packed
        xeV = np.zeros((XROWS, 2, V_pad), BF16)
        m = U_nodes >= 0
        xeV[:, :, np.nonzero(m)[0]] = slot_arrays(U_nodes[m], int(m.sum()))
        xeSV = slot_arrays(c["SVo"], n_SV_pad)
        xeH = slot_arrays(c["H"], H_pad)

        # conv2 gather idx + s2 flat [P, n_slots*WW], per (tile, dst-window)
        gat_idx = np.zeros(E2_slots, np.int64)
        s2f = np.zeros((P, n_slots * WW), np.float32)
        hpos = np.full(NN, -1, np.int64)
        hpos[c["H"]] = np.arange(len(c["H"]))
        vp = vpos[c["e_dst"]]
        eord = np.argsort(vp, kind="stable")
        bnd = np.searchsorted(vp[eord], np.arange(0, NT_V * P + 1, WW))
        st = 0
        for t in range(NT_V):
            for w in range(NWIN):
                nsub = int(wsub[t, w])
                if t < c["NT_V"]:
                    k = t * NWIN + w
                    sel = eord[bnd[k]:bnd[k + 1]]
                    es, ed, ew = c["e_src"][sel], c["e_dst"][sel], c["e_ea"][sel]
                    # gather position in table A: SV->apos, H->n_SV_pad+pos
                    ga = np.where(apos[es] >= 0, apos[es], n_SV_pad + hpos[es])
                    ne = len(es)
                    if ne:
                        assert (ga >= 0).all() and (ga < A_rows).all()
                        assert ne <= nsub * P
                        i = np.arange(ne)
                        # tabA is partition-major: packed pos n lives at
                        # virtual row (n%P)*NTB + n//P (one 8KB descriptor
                        # per partition on the batched conv1 writes)
                        gat_idx[st * P + i] = (ga % P) * (A_rows // P) + ga // P
                        s2f[i % P, (st + i // P) * WW
                            + (vpos[ed] - t * P - w * WW)] = ew
                st += nsub
        assert st == n_slots

        # pool S3 [NT_V, P, MAXG_TILE] 0/1, graph ids / inv counts
        s3 = np.zeros((NT_V, P, MAXG_TILE), np.float32)
        pool_gid = np.full((NT_V, MAXG_TILE), meta["GCOLS"] + 100, np.int64)
        for t in range(c["NT_V"]):
            gcols, gcnts = c["tile_graphs"][t]
            off = 0
            for j, (gc, n) in enumerate(zip(gcols, gcnts)):
                s3[t, off:off + n, j] = 1.0 / n
                pool_gid[t, j] = gc
                off += n

        # active-shot mask for decoder
        amask = np.zeros(meta["n_shot_core"], np.float32)
        amask[(shot_of[c["gids"]] // NC)] = 1.0

        c["arrays"] = dict(
            xeV=xeV, xeSV=xeSV, xeH=xeH, gat_idx=gat_idx, s2f=s2f.astype(BF16),
            s3=s3.astype(BF16), pool_gid=pool_gid, amask=amask,
        )
    return cores, meta


def wrap_idx16(idx, pad_to):
    """int idx array -> dma_gather int16 layout [128, pad_to//16], 0 padded."""
    a = np.zeros(pad_to, np.int16)
    a[:len(idx)] = idx.astype(np.int16)
    w = a.reshape(pad_to // 16, 16).T  # [16, C]
    return np.tile(w, (8, 1)).copy()


# ======================================================
import sys as _sys
if "/opt/trn_rl_repo" not in _sys.path:
    _sys.path.insert(0, "/opt/trn_rl_repo")
import concourse.bass as bass
import concourse.bacc as bacc
import concourse.mybir as mybir
from concourse.tile import TileContext


BF = mybir.dt.bfloat16
FP = mybir.dt.float32
AF = mybir.ActivationFunctionType


def build(meta, num_devices=8, stop_after="full", use_any=True, c2sub="all", sens=()):
    NT_V, NT_H = meta["NT_V"], meta["NT_H"]
    V_pad, H_pad = meta["V_pad"], meta["H_pad"]
    n_SV_pad, A_rows = meta["n_SV_pad"], meta["A_rows"]
    T_sub = meta["T_sub"]
    wsub = meta["wsub"]
    XR_V, XR_SV, XR_H = meta["XR_V"], meta["XR_SV"], meta["XR_H"]
    E2_slots = meta["E2_slots"]
    n_slots = E2_slots // P
    GCOLS, G_rows = meta["GCOLS"], meta["G_rows"]
    NSH = meta["n_shot_core"]
    NT_G = G_rows // P

    nc = bacc.Bacc("TRN2", target_bir_lowering=False, debug=False,
                   num_devices=num_devices)

    def inp(name, shape, dt):
        return nc.dram_tensor(name, shape, dt, kind="ExternalInput")

    xeV_d = inp("xeV", [XROWS, 2, V_pad], BF)
    xeSV_d = inp("xeSV", [XROWS, 2, n_SV_pad], BF)
    xeH_d = inp("xeH", [XROWS, 2, H_pad], BF)
    gat_d = inp("gat", [128, E2_slots // 16], mybir.dt.int16)
    s2_d = inp("s2", [P, n_slots * WW], BF)
    s3_d = inp("s3", [NT_V, P, MAXG_TILE], BF)
    pgid_d = inp("pgid", [NT_V // 4, P, 1], mybir.dt.int32)
    amask_d = inp("amask", [12, NSH], FP)
    ident_d = inp("ident", [P, P], BF)
    w1_d = inp("w1", [P, P], BF)           # rows 0..84 = w1full
    wrel2_d = inp("wrel2", [P, F2], BF)
    wroot2_d = inp("wroot2", [P, F2], BF)
    gruw_d = inp("gruw", [15, P, P], BF)   # wih0(6), whh0(3), wih1(3), whh1(3)
    dec_d = inp("dec", [P, 12], BF)
    out_d = nc.dram_tensor("out", [12, NSH], FP, kind="ExternalOutput")

    tabA_d = nc.dram_tensor("tabA", [A_rows, F1], BF, kind="Internal")
    emb_d = nc.dram_tensor("emb", [G_rows, F2], BF, kind="Internal")

    lvl = ("conv1", "conv1bb", "conv2", "full").index(stop_after) - 1

    NSL_V = NT_V // GRP
    NSL_SV = n_SV_pad // (GRP * P)
    NSL_H = NT_H // GRP
    SLAB = GRP * P   # 1024 columns per slab

    with TileContext(nc) as tc:
        with (
            tc.tile_pool(name="const", bufs=1) as cpool,
            tc.tile_pool(name="sb", bufs=3) as pool,
            tc.tile_pool(name="big", bufs=3) as bigp,
            tc.tile_pool(name="gi", bufs=1) as gip,
            tc.tile_pool(name="psA", bufs=2, space="PSUM") as psA,
            tc.tile_pool(name="psH", bufs=2, space="PSUM") as psH,
            tc.tile_pool(name="psP", bufs=1, space="PSUM") as psP,
            tc.tile_pool(name="psC", bufs=3, space="PSUM") as psC,
        ):
            anye = nc.any if use_any else nc.vector
            ident = cpool.tile([P, P], BF, tag="ident")
            nc.sync.dma_start(out=ident[:], in_=ident_d[:])
            w1 = cpool.tile([P, P], BF, tag="w1")
            nc.sync.dma_start(out=w1[:], in_=w1_d[:])
            wrel2 = cpool.tile([P, F2], BF, tag="wrel2")
            nc.scalar.dma_start(out=wrel2[:], in_=wrel2_d[:])
            wroot2 = cpool.tile([P, F2], BF, tag="wroot2")
            nc.scalar.dma_start(out=wroot2[:], in_=wroot2_d[:])
            # hresT: conv1 output [F1, node] for V tiles, SBUF-resident
            hresT = cpool.tile([P, V_pad], BF, tag="hresT")

            zt = cpool.tile([P, NT_G * F2], BF, tag="zero")
            nc.gpsimd.memset(zt[:], 0.0)
            nc.sync.dma_start(
                out=emb_d[:].rearrange("(b p) f -> p b f", p=P),
                in_=zt[:].rearrange("p (b f) -> p b f", f=F2))

            # ---------------- conv1 ----------------
            # MAC slabs per macro: one DMA load (and one tabA write) covers
            # MAC slabs -- DMA queue time is fixed-overhead dominated
            # (~1.5us/instruction), so batch aggressively.
            MAC = 4
            tabAv = tabA_d[:].rearrange("(p b) f -> p b f", p=P)

            def conv1_macro(src_d, s0, nsl, XRs, i, mode, dst_rows=None):
                """nsl slabs starting at slab s0: load, msg-mul, per-slab
                matmul+relu. mode 'A': -> hresT cols; 'B': -> tabA rows
                (partition-major virtual layout) at dst_rows."""
                xr = max(XRs[s0:s0 + nsl])
                c0 = s0 * SLAB
                CW = nsl * SLAB
                eng = nc.sync
                xe = pool.tile([XROWS, 2 * MAC * SLAB], BF, tag="xe")
                if "c1load" not in sens:
                    eng.dma_start(
                        out=xe[0:xr, 0:2 * CW].rearrange(
                            "p (two n) -> p two n", two=2),
                        in_=src_d[0:xr, :, c0:c0 + CW])
                msgT = pool.tile([XROWS, MAC * SLAB], BF, tag="msgT")
                if "c1mul" not in sens:
                    anye.tensor_mul(out=msgT[0:xr, 0:CW], in0=xe[0:xr, 0:CW],
                                    in1=xe[0:xr, CW:2 * CW])
                if mode == "A":
                    for half in range(CW // 512):
                        ps = psC.tile([P, 512], FP, tag="pC")
                        if "c1mm" not in sens:
                            nc.tensor.matmul(
                                ps[:], lhsT=w1[0:xr, :],
                                rhs=msgT[0:xr, half * 512:(half + 1) * 512],
                                start=True, stop=True)
                        if "c1relu" not in sens:
                            anye.tensor_relu(
                                out=hresT[:, c0 + half * 512:c0 + (half + 1) * 512],
                                in_=ps[:])
                else:
                    h1m = pool.tile([P, MAC * GRP * F1], BF, tag="h1m")
                    for half in range(CW // 512):
                        ps = psC.tile([P, 512], FP, tag="pC")
                        for j in range(4) if "c1mm" not in sens else []:
                            jj = half * 4 + j
                            nc.tensor.matmul(
                                ps[:, j * P:(j + 1) * P],
                                lhsT=msgT[0:xr, jj * P:(jj + 1) * P],
                                rhs=w1[0:xr, :], start=True, stop=True)
                        if "c1relu" not in sens:
                            anye.tensor_relu(
                                out=h1m[:, half * 512:(half + 1) * 512],
                                in_=ps[:])
                    # one write: 128 descriptors x nb*256B contiguous
                    wr_eng = (nc.gpsimd, nc.sync)[i % 2]
                    nb = CW // P
                    B0 = dst_rows // P
                    if "c1wr" not in sens:
                        wr_eng.dma_start(
                            out=tabAv[:, B0:B0 + nb, :],
                            in_=h1m[:, 0:nb * F1].rearrange(
                                "p (b f) -> p b f", f=F1))

            def macro_list(nsl_tot):
                out = []
                s = 0
                while s < nsl_tot:
                    out.append((s, min(MAC, nsl_tot - s)))
                    s += MAC
                return out

            mi = 0
            for (s0, nsl) in macro_list(NSL_SV):
                conv1_macro(xeSV_d, s0, nsl, XR_SV, mi, "B",
                            dst_rows=s0 * SLAB)
                mi += 1
            for (s0, nsl) in macro_list(NSL_H):
                conv1_macro(xeH_d, s0, nsl, XR_H, mi, "B",
                            dst_rows=n_SV_pad + s0 * SLAB)
                mi += 1

            tc.strict_bb_all_engine_barrier()
            # V macros only write SBUF (hresT) -- emitted after the barrier,
            # interleaved with conv2 groups that consume them
            vdone = 0
            vmi = 0
            if lvl < 1:
                for (s0, nsl) in macro_list(NSL_V):
                    conv1_macro(xeV_d, s0, nsl, XR_V, s0 // MAC, "A")
            if stop_after == "conv1bb":
                tc.strict_bb_all_engine_barrier()

            # ---------------- conv2 + pool ----------------
            if lvl >= 1:
                gat_t = cpool.tile([128, E2_slots // 16], mybir.dt.int16,
                                   tag="gat")
                nc.sync.dma_start(out=gat_t[:], in_=gat_d[:])
                s3all = cpool.tile([P, NT_V * MAXG_TILE], BF, tag="s3all")
                nc.scalar.dma_start(
                    out=s3all[:].rearrange("p (t g) -> p t g", g=MAXG_TILE),
                    in_=s3_d[:].rearrange("t p g -> p t g"))
                pgall = cpool.tile([P, NT_V // 4], mybir.dt.int32, tag="pgall")
                nc.sync.dma_start(
                    out=pgall[:].rearrange("p (b one) -> p b one", one=1),
                    in_=pgid_d[:].rearrange("b p one -> p b one"))
                sub_start = np.concatenate([[0], np.cumsum(T_sub)]).astype(int)
                plan = []
                t = 0
                while t < NT_V:
                    te = t
                    while te < NT_V and sub_start[te + 1] - sub_start[t] <= GG:
                        te += 1
                    plan.append((t, te))
                    t = te
                pool_ps = None
                tsrc = {}           # tile -> (gt, s2sl, so)
                assert NT_V % 2 == 0

                def emit_pair(t0):
                    # two tiles share one agg psum bank + one h2 psum bank,
                    # halving the PSUM->SBUF copy / relu op count
                    aggp = psA.tile([P, 2 * P], FP, tag="pA")
                    for q in range(2):
                        gt, s2sl, so = tsrc.pop(t0 + q)
                        s = 0
                        for w in range(NWIN):
                            nw_ = wsub[t0 + q][w]
                            for k in range(nw_):
                                nc.tensor.matmul(
                                    aggp[:, q * P + w * WW:
                                         q * P + (w + 1) * WW],
                                    lhsT=gt[:, so + s, :],
                                    rhs=s2sl[:, (so + s) * WW:
                                             (so + s + 1) * WW],
                                    start=(k == 0), stop=(k == nw_ - 1))
                                s += 1
                    aggs = pool.tile([P, 2 * P], BF, tag="agg2Ts")
                    anye.tensor_copy(out=aggs[:], in_=aggp[:])
                    h2p = psH.tile([P, 2 * F2], FP, tag="pB")
                    for q in range(2):
                        t = t0 + q
                        nc.tensor.matmul(h2p[:, q * F2:(q + 1) * F2],
                                         lhsT=aggs[:, q * P:(q + 1) * P],
                                         rhs=wrel2[:], start=True, stop=False)
                        nc.tensor.matmul(h2p[:, q * F2:(q + 1) * F2],
                                         lhsT=hresT[:, t * P:(t + 1) * P],
                                         rhs=wroot2[:], start=False, stop=True)
                    h2s = pool.tile([P, 2 * F2], BF, tag="h2s")
                    anye.tensor_relu(out=h2s[:], in_=h2p[:])
                    for q in range(2):
                        t = t0 + q
                        jj = t % 4
                        if jj == 0:
                            pp = psP.tile([P, F2], FP, tag="pP")
                            pool_ps[0] = pp
                        nc.tensor.matmul(
                            pool_ps[0][32 * jj:32 * jj + 32, :],
                            lhsT=s3all[:, t * MAXG_TILE:(t + 1) * MAXG_TILE],
                            rhs=h2s[:, q * F2:(q + 1) * F2],
                            start=True, stop=True, tile_position=(0, 32 * jj))
                        if jj == 3 or t == NT_V - 1:
                            npart = 32 * (jj + 1)
                            pls = pool.tile([P, F2], BF, tag="pls")
                            anye.tensor_copy(out=pls[:npart, :],
                                               in_=pool_ps[0][:npart, :])
                            nc.gpsimd.indirect_dma_start(
                                out=emb_d[:, :],
                                out_offset=bass.IndirectOffsetOnAxis(
                                    ap=pgall[:npart, t // 4:t // 4 + 1], axis=0),
                                in_=pls[:npart, :], in_offset=None,
                                bounds_check=GCOLS, oob_is_err=False)

                pool_ps = [None]
                for gi_, (ta, te) in enumerate(plan):
                    while vdone * GRP < te:
                        nsl = min(MAC, NSL_V - vdone)
                        conv1_macro(xeV_d, vdone, nsl, XR_V, vmi, "A")
                        vdone += nsl
                        vmi += 1
                    ns = int(sub_start[te] - sub_start[ta])
                    gt = bigp.tile([P, GG, F1], BF, tag="g2")
                    # SWDGE ring holds 1024 descs -> <=8 subtiles/gather
                    for q0 in range(0, ns, 8):
                        qn = min(8, ns - q0)
                        a8 = int(sub_start[ta]) + q0
                        nc.gpsimd.dma_gather(
                            gt[:, q0:q0 + qn, :], tabA_d[:],
                            gat_t[:, a8 * 8:(a8 + qn) * 8],
                            qn * P, qn * P, F1)
                    s2sl = bigp.tile([P, GG * WW], BF, tag="s2sl")
                    (nc.sync, nc.gpsimd)[gi_ % 2].dma_start(
                        out=s2sl[:, :ns * WW],
                        in_=s2_d[:, int(sub_start[ta]) * WW:int(sub_start[te]) * WW])
                    for t in range(ta, te):
                        tsrc[t] = (gt, s2sl, int(sub_start[t] - sub_start[ta]))
                        if t % 2 == 1:
                            emit_pair(t - 1)

                while vdone < NSL_V:
                    nsl = min(MAC, NSL_V - vdone)
                    conv1_macro(xeV_d, vdone, nsl, XR_V, vmi, "A")
                    vdone += nsl
                    vmi += 1
                tc.strict_bb_all_engine_barrier()

            # ---------------- GRU (round-major emb pipeline) ----------------
            if lvl >= 2:
                gruw = cpool.tile([P, 15 * P], BF, tag="gruw")
                nc.sync.dma_start(
                    out=gruw[:].rearrange("p (w q) -> p w q", w=15),
                    in_=gruw_d[:].rearrange("w p q -> p w q"))
                dec = cpool.tile([P, 12], BF, tag="dec")
                nc.scalar.dma_start(out=dec[:], in_=dec_d[:])
                am = cpool.tile([12, NSH], FP, tag="am")
                nc.scalar.dma_start(out=am[:], in_=amask_d[:])

                # gruw cols: wih0 g0k0,g0k1,g1k0,g1k1,g2k0,g2k1 | whh0 x3
                #            | wih1 x3 | whh1 x3
                def gw(i):
                    return gruw[:, i * P:(i + 1) * P]

                wih0 = [gw(i) for i in range(6)]
                whh0 = [gw(6 + i) for i in range(3)]
                wih1 = [gw(9 + i) for i in range(3)]
                whh1 = [gw(12 + i) for i in range(3)]

                h0 = cpool.tile([P, NSH], BF, tag="h_L0")
                nc.gpsimd.memset(h0[:], 0.0)
                h1 = cpool.tile([P, NSH], BF, tag="h_L1")
                nc.gpsimd.memset(h1[:], 0.0)

                def gates_to_h(ps, gin_n, h, nm):
                    """ps cols [0:2N] hold r|z pre-activations (summed),
                    [2N:3N] gh_n; gin_n = gi n-gate AP. Updates h."""
                    rz = pool.tile([P, 2 * NSH], BF, tag=f"rz{nm}")
                    nc.scalar.activation(rz[:], ps[:, 0:2 * NSH], AF.Sigmoid)
                    ns_ = pool.tile([P, NSH], BF, tag=f"ns{nm}")
                    anye.tensor_mul(out=ns_[:], in0=rz[:, 0:NSH],
                                    in1=ps[:, 2 * NSH:3 * NSH])
                    anye.tensor_add(out=ns_[:], in0=ns_[:], in1=gin_n)
                    nc.scalar.activation(ns_[:], ns_[:], AF.Tanh)
                    hmn = pool.tile([P, NSH], BF, tag=f"hmn{nm}")
                    anye.tensor_sub(out=hmn[:], in0=h[:], in1=ns_[:])
                    anye.tensor_mul(out=hmn[:], in0=hmn[:],
                                    in1=rz[:, NSH:2 * NSH])
                    anye.tensor_add(out=h[:], in0=ns_[:], in1=hmn[:])

                for t in range(TR):
                    # load + transpose round-t embeddings: xt [feat-half, k, shot]
                    et = pool.tile([P, F2], BF, tag="et")
                    nc.sync.dma_start(out=et[:], in_=emb_d[t * P:(t + 1) * P, :])
                    xt = pool.tile([P, 2, P], BF, tag="xt")
                    for half in range(2):
                        tp = psA.tile([P, P], FP, tag="pA")
                        nc.tensor.matmul(tp[:], lhsT=et[:, half * P:(half + 1) * P],
                                         rhs=ident[:], start=True, stop=True)
                        anye.tensor_copy(out=xt[:, half, :], in_=tp[:])

                    # L0 step t: psum regions [r|z] = wih0@x_t + whh0@h0,
                    # [2N:3N] = whh0_n@h0, [3N:4N] = wih0_n@x_t
                    ps0 = psC.tile([P, 512], FP, tag="pC")
                    for gate in range(2):
                        reg = ps0[:, gate * NSH:(gate + 1) * NSH]
                        nc.tensor.matmul(reg, lhsT=whh0[gate], rhs=h0[:],
                                         start=True, stop=False)
                        nc.tensor.matmul(reg, lhsT=wih0[gate * 2], rhs=xt[:, 0, :],
                                         start=False, stop=False)
                        nc.tensor.matmul(reg, lhsT=wih0[gate * 2 + 1],
                                         rhs=xt[:, 1, :], start=False, stop=True)
                    nc.tensor.matmul(ps0[:, 2 * NSH:3 * NSH], lhsT=whh0[2],
                                     rhs=h0[:], start=True, stop=True)
                    nc.tensor.matmul(ps0[:, 3 * NSH:4 * NSH], lhsT=wih0[4],
                                     rhs=xt[:, 0, :], start=True, stop=False)
                    nc.tensor.matmul(ps0[:, 3 * NSH:4 * NSH], lhsT=wih0[5],
                                     rhs=xt[:, 1, :], start=False, stop=True)
                    gates_to_h(ps0, ps0[:, 3 * NSH:4 * NSH], h0, "0")

                    # L1 step t
                    ps1 = psC.tile([P, 512], FP, tag="pC")
                    for gate in range(2):
                        reg = ps1[:, gate * NSH:(gate + 1) * NSH]
                        nc.tensor.matmul(reg, lhsT=wih1[gate], rhs=h0[:],
                                         start=True, stop=False)
                        nc.tensor.matmul(reg, lhsT=whh1[gate], rhs=h1[:],
                                         start=False, stop=True)
                    nc.tensor.matmul(ps1[:, 2 * NSH:3 * NSH], lhsT=whh1[2],
                                     rhs=h1[:], start=True, stop=True)
                    nc.tensor.matmul(ps1[:, 3 * NSH:4 * NSH], lhsT=wih1[2],
                                     rhs=h0[:], start=True, stop=True)
                    gates_to_h(ps1, ps1[:, 3 * NSH:4 * NSH], h1, "1")

                hlast = h1
                lp = psA.tile([P, P], FP, tag="pA")
                nc.tensor.matmul(lp[:12, :NSH], lhsT=dec[:], rhs=hlast[:],
                                 start=True, stop=True)
                lo = pool.tile([12, NSH], FP, tag="lo")
                nc.vector.tensor_mul(out=lo[:], in0=lp[:12, :NSH], in1=am[:])
                nc.sync.dma_start(out=out_d[:], in_=lo[:])

            else:
                lo = pool.tile([12, NSH], FP, tag="lo")
                nc.gpsimd.memset(lo[:], 0.0)
                nc.sync.dma_start(out=out_d[:], in_=lo[:])

    nc.compile()
    return nc


def make_in_map(c, meta, W):
    """Per-core input arrays for run_bass_kernel_spmd."""
    A = c["arrays"]
    bf = lambda a: np.ascontiguousarray(a, dtype=BF16)
    f32 = lambda a: np.ascontiguousarray(a, dtype=np.float32)

    # w1full rows: slot*5+f -> wrel[f]; 80+f -> wroot[f]
    w1 = np.zeros((P, P), np.float32)
    w1[:F_IN] = f32(W["c1_wroot"])
    w1[F_IN:XROWS] = np.tile(f32(W["c1_wrel"]), (KSLOT, 1))
    wih0 = [f32(W["w_ih0"])[g * P:(g + 1) * P, k * P:(k + 1) * P].T
            for g in range(3) for k in range(2)]
    whh0 = [f32(W["w_hh0"])[g * P:(g + 1) * P, :].T for g in range(3)]
    wih1 = [f32(W["w_ih1"])[g * P:(g + 1) * P, :].T for g in range(3)]
    whh1 = [f32(W["w_hh1"])[g * P:(g + 1) * P, :].T for g in range(3)]
    gruw = np.stack(wih0 + whh0 + wih1 + whh1)
    amask = np.broadcast_to(A["amask"][None, :], (12, meta["n_shot_core"]))

    return {
        "xeV": bf(A["xeV"]),
        "xeSV": bf(A["xeSV"]),
        "xeH": bf(A["xeH"]),
        "gat": np.ascontiguousarray(wrap_idx16(A["gat_idx"], meta["E2_slots"])),
        "s2": bf(A["s2f"]),
        "s3": bf(A["s3"]),
        "pgid": np.ascontiguousarray(
            A["pool_gid"].reshape(-1, P, 1), dtype=np.int32),
        "amask": f32(amask),
        "ident": bf(np.eye(P, dtype=np.float32)),
        "w1": bf(w1),
        "wrel2": bf(W["c2_wrel"]),
        "wroot2": bf(W["c2_wroot"]),
        "gruw": bf(gruw),
        "dec": bf(W["dec_w"]),
    }


# ------------------------------------------------------------------
_CACHE = {}


def _get_nc(meta):
    key = (meta["NT_V"], meta["NT_H"], meta["n_SV_pad"], meta["E2_slots"],
           meta["G_rows"], tuple(meta["T_sub"]),
           tuple(tuple(r) for r in meta["wsub"]),
           tuple(meta["XR_V"]), tuple(meta["XR_SV"]), tuple(meta["XR_H"]))
    if key not in _CACHE:
        _CACHE[key] = build(meta, num_devices=NC)
    return _CACHE[key]


def kernel(**inputs):
    import sys as _sys
    if "/opt/trn_rl_repo" not in _sys.path:
        _sys.path.insert(0, "/opt/trn_rl_repo")
    from concourse.bass_utils import run_bass_kernel_spmd

    for k in ("c1_b", "c2_b", "b_ih0", "b_hh0", "b_ih1", "b_hh1", "dec_b",
              "empty_emb"):
        assert not np.any(np.asarray(inputs[k])), f"nonzero {k} unsupported"

    cores, meta = prep(inputs)
    W = {k: np.asarray(v, np.float32) for k, v in inputs.items()
         if k not in ("x", "edge_index", "edge_attr", "batch_labels",
                      "label_map", "B")}
    nc = _get_nc(meta)
    in_maps = [make_in_map(c, meta, W) for c in cores]
    res = None
    for attempt in range(4):
        try:
            res = run_bass_kernel_spmd(nc, in_maps, core_ids=list(range(NC)))
            break
        except Exception:
            if attempt == 3:
                raise
    B = meta["B"]
    out = np.zeros((B, 12), np.float32)
    nsh = meta["n_shot_core"]
    for d in range(NC):
        lg = res.results[d]["out"]          # [12, nsh]
        s = d + NC * np.arange(nsh)
        out[s[s < B]] = lg.T[s < B]
    return out



# revision 19
# speedup vs baseline: 1.1018x; 1.1018x over previous
"""Host-side sharding/prep + Bass device program for nn_BBGRUDecoder.

Host code does index manipulation / data layout only (no model FLOPs);
the device kernel does all arithmetic.

Device-side structure (per core, SPMD over 8 cores; shots sharded
round-robin by shot % 8):

  conv1: node features staged transposed as [85, U_pad] bf16 where rows
    0..79 are the 16 in-edge slots x 5 features and rows 80..84 are the
    node's own features (eav rows are edge weights, 1.0 for the root
    block). msg = xs*eav elementwise, then ONE matmul per tile against
    w1full [85,128] (wrel tiled 16x + wroot) folds the slot reduction,
    the root term and the F1 projection. V tiles produce hresT [F1,node]
    (SBUF-resident, feeds conv2's root term); SV + halo tiles produce
    [node,F1] rows written to the DRAM gather table tabA.
  conv2: per 128-edge subtile, gather source rows from tabA (Pool SWDGE)
    and matmul against the sparse scatter matrix s2 (flat [128,E2_slots]
    layout for full-rate slab loads). Root term from hresT. Pool by
    graph via s3 0/1 matrices, scatter per-graph sums to emb (indirect
    DMA, graph-column ids).
  GRU: emb -> embT [128, GCOLS], batched input projections, 10-step
    recurrence x 2 layers, decoder matmul, active-shot mask.
"""
import numpy as np
import ml_dtypes

BF16 = np.dtype(ml_dtypes.bfloat16)
NC = 8
P = 128
KSLOT = 16       # conv1 in-edge slots per node (max in-degree 13)
F_IN = 5
F1 = 128
F2 = 256
HID = 128
TR = 10          # rounds per shot
MAXG_TILE = 32   # max graphs per node-tile (pool S3 width)
IDX_CAP = 32767
XROWS = KSLOT * F_IN + F_IN   # 85: root rows (first 5) + slot rows
GRP = 8          # node tiles per conv1 slab
GG = 16          # conv2 edge subtiles per gather group
WW = 32          # conv2 dst-window width (s2 block cols)
NWIN = P // WW   # dst windows per node tile


def _pack_groups(sizes, cap_items, cap_groups):
    """Greedy-pack consecutive groups (each <=cap_items items) into tiles of
    <=cap_items items and <=cap_groups groups. Returns list of (start_group,
    n_groups, n_items)."""
    tiles = []
    i = 0
    n = len(sizes)
    while i < n:
        items = 0
        g = 0
        while i + g < n and g < cap_groups and items + sizes[i + g] <= cap_items:
            items += sizes[i + g]
            g += 1
        assert g > 0, f"group {i} size {sizes[i]} exceeds cap {cap_items}"
        tiles.append((i, g, items))
        i += g
    return tiles


def prep(inputs):
    x = np.asarray(inputs["x"], np.float32)
    ei = np.asarray(inputs["edge_index"], np.int64)
    ea = np.asarray(inputs["edge_attr"], np.float32)
    bl = np.asarray(inputs["batch_labels"], np.int64)
    lm = np.asarray(inputs["label_map"], np.int64)
    B = int(inputs["B"])
    NN = x.shape[0]
    src_g, dst_g = ei[0], ei[1]
    shot_of, round_of = lm[:, 0], lm[:, 1]
    n_shot_core = (B + NC - 1) // NC          # 128 shots per core
    GCOLS = n_shot_core * TR                  # 1280 graph-columns per core
    deg = np.bincount(dst_g, minlength=NN)
    assert deg.max() <= KSLOT

    node_g = bl
    node_core = (shot_of[node_g] % NC).astype(np.int64)

    cores = []
    for d in range(NC):
        V = np.nonzero(node_core == d)[0]          # ascending node ids
        gids, gstart, gcnt = np.unique(node_g[V], return_index=True, return_counts=True)
        # order graphs by max node in-degree so slab XR (=5+5*maxdeg) shrinks
        gmax = np.maximum.reduceat(deg[V], gstart)
        gord = np.argsort(gmax, kind="stable")
        gids, gstart, gcnt = gids[gord], gstart[gord], gcnt[gord]
        # graph column within core: s_idx*TR + round
        s_idx = shot_of[gids] // NC
        # round-major: emb tile t holds exactly the round-t embeddings,
        # so the GRU consumes emb tiles in step order (pipelines with loads)
        gcol = round_of[gids] * n_shot_core + s_idx
        # ---- graph-aligned node tiles ----
        tiles = _pack_groups(gcnt.tolist(), P, MAXG_TILE)
        NT_V = len(tiles)
        vpos = np.full(NN, -1, np.int64)        # global node id -> packed pos
        packed_rows = []                        # per tile: global node ids (len<=P)
        tile_graphs = []                        # per tile: (gcol list, counts)
        for (g0, ng, ni) in tiles:
            rows = [V[gstart[k]:gstart[k] + gcnt[k]] for k in range(g0, g0 + ng)]
            rows = np.concatenate(rows)
            packed_rows.append(rows)
            tile_graphs.append((gcol[g0:g0 + ng], gcnt[g0:g0 + ng]))
        for t, rows in enumerate(packed_rows):
            vpos[rows] = t * P + np.arange(len(rows))

        # ---- conv2 edges (dst in V) ----
        E = np.nonzero(node_core[dst_g] == d)[0]
        e_src, e_dst, e_ea = src_g[E], dst_g[E], ea[E]
        S = np.unique(e_src)
        SV = S[node_core[S] == d]
        H = S[node_core[S] != d]
        H = H[np.argsort(deg[H], kind="stable")]   # degree-sorted slabs
        # table A (gather source) layout: [SV (degree-sorted), H]
        SVo = SV[np.argsort(deg[SV], kind="stable")]
        apos = np.full(NN, -1, np.int64)
        apos[SVo] = np.arange(len(SVo))
        cores.append(dict(
            d=d, V=V, NT_V=NT_V, tiles=tiles, packed_rows=packed_rows,
            tile_graphs=tile_graphs, vpos=vpos,
            E=E, e_src=e_src, e_dst=e_dst, e_ea=e_ea,
            SVo=SVo, H=H, apos=apos, gids=gids, gcol=gcol, gcnt=gcnt,
        ))

    # ---- shared static shapes ----
    NT_V = max(c["NT_V"] for c in cores)
    NT_V = -(-NT_V // GRP) * GRP              # pad to slab multiple
    n_SV_pad = -(-max(len(c["SVo"]) for c in cores) // (GRP * P)) * (GRP * P)
    NT_H = -(-max(len(c["H"]) for c in cores) // P)
    NT_H = -(-NT_H // GRP) * GRP
    H_pad = NT_H * P
    A_rows = n_SV_pad + H_pad
    assert A_rows <= IDX_CAP + 1, A_rows
    V_pad = NT_V * P

    # conv2: per-tile x per-dst-window edge subtile counts (max over cores).
    # Each window covers WW consecutive dst slots; a subtile is <=128 edges
    # whose dsts fall in that window -> s2 block is [128, WW] not [128, 128].
    wsub = np.ones((NT_V, NWIN), np.int64)
    for c in cores:
        vp = c["vpos"][c["e_dst"]]
        cnt = np.bincount(vp // WW, minlength=NT_V * NWIN)[:NT_V * NWIN]
        ns = np.maximum(-(-cnt // P), 1).reshape(-1, NWIN)
        n_t = ns.shape[0]
        wsub[:n_t] = np.maximum(wsub[:n_t], ns)
    T_sub = wsub.sum(axis=1)
    n_slots = int(T_sub.sum())
    E2_slots = n_slots * P

    # per-slab conv1 staging rows: XR = 5 (root) + 5*max_indegree_in_slab
    NSL_V_ = NT_V // GRP
    NSL_SV_ = n_SV_pad // (GRP * P)
    NSL_H_ = NT_H // GRP
    KV = np.zeros(NSL_V_, np.int64)
    KSV = np.zeros(NSL_SV_, np.int64)
    KH = np.zeros(NSL_H_, np.int64)
    for c in cores:
        for s in range(NSL_V_):
            rows = [c["packed_rows"][t]
                    for t in range(s * GRP, min((s + 1) * GRP, c["NT_V"]))]
            rows = np.concatenate(rows) if rows else np.zeros(0, np.int64)
            if len(rows):
                KV[s] = max(KV[s], deg[rows].max())
        for s in range(NSL_SV_):
            rows = c["SVo"][s * GRP * P:(s + 1) * GRP * P]
            if len(rows):
                KSV[s] = max(KSV[s], deg[rows].max())
        for s in range(NSL_H_):
            rows = c["H"][s * GRP * P:(s + 1) * GRP * P]
            if len(rows):
                KH[s] = max(KH[s], deg[rows].max())
    XR_V = (F_IN + F_IN * KV).tolist()
    XR_SV = (F_IN + F_IN * KSV).tolist()
    XR_H = (F_IN + F_IN * KH).tolist()

    meta = dict(NT_V=NT_V, NT_H=NT_H, V_pad=V_pad, H_pad=H_pad,
                n_SV_pad=n_SV_pad, A_rows=A_rows, T_sub=T_sub.tolist(),
                wsub=wsub.tolist(), XR_V=XR_V, XR_SV=XR_SV, XR_H=XR_H,
                E2_slots=E2_slots, GCOLS=GCOLS, G_rows=-(-(GCOLS + 1) // P) * P,
                n_shot_core=n_shot_core, B=B)

    # ---- per-core padded arrays ----
    ones5 = np.ones(F_IN, np.float32)
    for c in cores:
        vpos, apos = c["vpos"], c["apos"]

        def slot_arrays(nodes, npad):
            """Transposed conv1 staging [XROWS, 2, npad] for the given
            node list (bf16): [:,0]=feature values, [:,1]=edge weights."""
            xs = np.zeros((npad, KSLOT, F_IN), np.float32)
            ev = np.zeros((npad, KSLOT, F_IN), np.float32)
            pos = np.full(NN, -1, np.int64)
            pos[nodes] = np.arange(len(nodes))
            # in-edges of these nodes from FULL edge list
            EU = np.nonzero(pos[dst_g] >= 0)[0]
            du = pos[dst_g[EU]]
            order = np.argsort(du, kind="stable")
            EU, du = EU[order], du[order]
            # vectorized running slot index within each dst group
            uniq, first = np.unique(du, return_index=True)
            sl = np.arange(len(EU)) - np.repeat(first, np.diff(
                np.concatenate([first, [len(EU)]])))
            xs[du, sl] = x[src_g[EU]]
            ev[du, sl] = ea[EU][:, None] * ones5
            out = np.zeros((XROWS, 2, npad), np.float32)
            out[:F_IN, 0, :len(nodes)] = x[nodes].T
            out[:F_IN, 1, :len(nodes)] = 1.0
            out[F_IN:, 0] = xs.reshape(npad, -1).T
            out[F_IN:, 1] = ev.reshape(npad, -1).T
            return out.astype(BF16)

        U_nodes = np.full(V_pad, -1, np.int64)
        for t, rows in enumerate(c["packed_rows"]):
            U_nodes[t * P:t * P + len(rows)] = rows
        # U_nodes has -1 gaps between tiles; compute compact, scatter to packed
        xeV = np.zeros((XROWS, 2, V_pad), BF16)
        m = U_nodes >= 0
        xeV[:, :, np.nonzero(m)[0]] = slot_arrays(U_nodes[m], int(m.sum()))
        xeSV = slot_arrays(c["SVo"], n_SV_pad)
        xeH = slot_arrays(c["H"], H_pad)

        # conv2 gather idx + s2 flat [P, n_slots*WW], per (tile, dst-window)
        gat_idx = np.zeros(E2_slots, np.int64)
        s2f = np.zeros((P, n_slots * WW), np.float32)
        hpos = np.full(NN, -1, np.int64)
        hpos[c["H"]] = np.arange(len(c["H"]))
        vp = vpos[c["e_dst"]]
        eord = np.argsort(vp, kind="stable")
        bnd = np.searchsorted(vp[eord], np.arange(0, NT_V * P + 1, WW))
        st = 0
        for t in range(NT_V):
            for w in range(NWIN):
                nsub = int(wsub[t, w])
                if t < c["NT_V"]:
                    k = t * NWIN + w
                    sel = eord[bnd[k]:bnd[k + 1]]
                    es, ed, ew = c["e_src"][sel], c["e_dst"][sel], c["e_ea"][sel]
                    # gather position in table A: SV->apos, H->n_SV_pad+pos
                    ga = np.where(apos[es] >= 0, apos[es], n_SV_pad + hpos[es])
                    ne = len(es)
                    if ne:
                        assert (ga >= 0).all() and (ga < A_rows).all()
                        assert ne <= nsub * P
                        i = np.arange(ne)
                        # tabA is partition-major: packed pos n lives at
                        # virtual row (n%P)*NTB + n//P (one 8KB descriptor
                        # per partition on the batched conv1 writes)
                        gat_idx[st * P + i] = (ga % P) * (A_rows // P) + ga // P
                        s2f[i % P, (st + i // P) * WW
                            + (vpos[ed] - t * P - w * WW)] = ew
                st += nsub
        assert st == n_slots

        # pool S3 [NT_V, P, MAXG_TILE] 0/1, graph ids / inv counts
        s3 = np.zeros((NT_V, P, MAXG_TILE), np.float32)
        pool_gid = np.full((NT_V, MAXG_TILE), meta["GCOLS"] + 100, np.int64)
        for t in range(c["NT_V"]):
            gcols, gcnts = c["tile_graphs"][t]
            off = 0
            for j, (gc, n) in enumerate(zip(gcols, gcnts)):
                s3[t, off:off + n, j] = 1.0 / n
                pool_gid[t, j] = gc
                off += n

        # active-shot mask for decoder
        amask = np.zeros(meta["n_shot_core"], np.float32)
        amask[(shot_of[c["gids"]] // NC)] = 1.0

        c["arrays"] = dict(
            xeV=xeV, xeSV=xeSV, xeH=xeH, gat_idx=gat_idx, s2f=s2f.astype(BF16),
            s3=s3.astype(BF16), pool_gid=pool_gid, amask=amask,
        )
    return cores, meta


def wrap_idx16(idx, pad_to):
    """int idx array -> dma_gather int16 layout [128, pad_to//16], 0 padded."""
    a = np.zeros(pad_to, np.int16)
    a[:len(idx)] = idx.astype(np.int16)
    w = a.reshape(pad_to // 16, 16).T  # [16, C]
    return np.tile(w, (8, 1)).copy()


# ======================================================
import sys as _sys
if "/opt/trn_rl_repo" not in _sys.path:
    _sys.path.insert(0, "/opt/trn_rl_repo")
import concourse.bass as bass
import concourse.bacc as bacc
import concourse.mybir as mybir
from concourse.tile import TileContext


BF = mybir.dt.bfloat16
FP = mybir.dt.float32
AF = mybir.ActivationFunctionType


def build(meta, num_devices=8, stop_after="full", use_any=True, c2sub="all", sens=()):
    NT_V, NT_H = meta["NT_V"], meta["NT_H"]
    V_pad, H_pad = meta["V_pad"], meta["H_pad"]
    n_SV_pad, A_rows = meta["n_SV_pad"], meta["A_rows"]
    T_sub = meta["T_sub"]
    wsub = meta["wsub"]
    XR_V, XR_SV, XR_H = meta["XR_V"], meta["XR_SV"], meta["XR_H"]
    E2_slots = meta["E2_slots"]
    n_slots = E2_slots // P
    GCOLS, G_rows = meta["GCOLS"], meta["G_rows"]
    NSH = meta["n_shot_core"]
    NT_G = G_rows // P

    nc = bacc.Bacc("TRN2", target_bir_lowering=False, debug=False,
                   num_devices=num_devices)

    def inp(name, shape, dt):
        return nc.dram_tensor(name, shape, dt, kind="ExternalInput")

    xeV_d = inp("xeV", [XROWS, 2, V_pad], BF)
    xeSV_d = inp("xeSV", [XROWS, 2, n_SV_pad], BF)
    xeH_d = inp("xeH", [XROWS, 2, H_pad], BF)
    gat_d = inp("gat", [128, E2_slots // 16], mybir.dt.int16)
    s2_d = inp("s2", [P, n_slots * WW], BF)
    s3_d = inp("s3", [NT_V, P, MAXG_TILE], BF)
    pgid_d = inp("pgid", [NT_V // 4, P, 1], mybir.dt.int32)
    amask_d = inp("amask", [12, NSH], FP)
    ident_d = inp("ident", [P, P], BF)
    w1_d = inp("w1", [P, P], BF)           # rows 0..84 = w1full
    wrel2_d = inp("wrel2", [P, F2], BF)
    wroot2_d = inp("wroot2", [P, F2], BF)
    gruw_d = inp("gruw", [15, P, P], BF)   # wih0(6), whh0(3), wih1(3), whh1(3)
    dec_d = inp("dec", [P, 12], BF)
    out_d = nc.dram_tensor("out", [12, NSH], FP, kind="ExternalOutput")

    tabA_d = nc.dram_tensor("tabA", [A_rows, F1], BF, kind="Internal")
    emb_d = nc.dram_tensor("emb", [G_rows, F2], BF, kind="Internal")

    lvl = ("conv1", "conv1bb", "conv2", "full").index(stop_after) - 1

    NSL_V = NT_V // GRP
    NSL_SV = n_SV_pad // (GRP * P)
    NSL_H = NT_H // GRP
    SLAB = GRP * P   # 1024 columns per slab

    with TileContext(nc) as tc:
        with (
            tc.tile_pool(name="const", bufs=1) as cpool,
            tc.tile_pool(name="sb", bufs=3) as pool,
            tc.tile_pool(name="big", bufs=3) as bigp,
            tc.tile_pool(name="gi", bufs=1) as gip,
            tc.tile_pool(name="psA", bufs=2, space="PSUM") as psA,
            tc.tile_pool(name="psH", bufs=2, space="PSUM") as psH,
            tc.tile_pool(name="psP", bufs=1, space="PSUM") as psP,
            tc.tile_pool(name="psC", bufs=3, space="PSUM") as psC,
        ):
            anye = nc.any if use_any else nc.vector
            ident = cpool.tile([P, P], BF, tag="ident")
            nc.sync.dma_start(out=ident[:], in_=ident_d[:])
            w1 = cpool.tile([P, P], BF, tag="w1")
            nc.sync.dma_start(out=w1[:], in_=w1_d[:])
            wrel2 = cpool.tile([P, F2], BF, tag="wrel2")
            nc.scalar.dma_start(out=wrel2[:], in_=wrel2_d[:])
            wroot2 = cpool.tile([P, F2], BF, tag="wroot2")
            nc.scalar.dma_start(out=wroot2[:], in_=wroot2_d[:])
            # hresT: conv1 output [F1, node] for V tiles, SBUF-resident
            hresT = cpool.tile([P, V_pad], BF, tag="hresT")

            zt = cpool.tile([P, NT_G * F2], BF, tag="zero")
            nc.gpsimd.memset(zt[:], 0.0)
            nc.sync.dma_start(
                out=emb_d[:].rearrange("(b p) f -> p b f", p=P),
                in_=zt[:].rearrange("p (b f) -> p b f", f=F2))

            # ---------------- conv1 ----------------
            # MAC slabs per macro: one DMA load (and one tabA write) covers
            # MAC slabs -- DMA queue time is fixed-overhead dominated
            # (~1.5us/instruction), so batch aggressively.
            MAC = 4
            tabAv = tabA_d[:].rearrange("(p b) f -> p b f", p=P)

            def conv1_macro(src_d, s0, nsl, XRs, i, mode, dst_rows=None):
                """nsl slabs starting at slab s0: load, msg-mul, per-slab
                matmul+relu. mode 'A': -> hresT cols; 'B': -> tabA rows
                (partition-major virtual layout) at dst_rows."""
                xr = max(XRs[s0:s0 + nsl])
                c0 = s0 * SLAB
                CW = nsl * SLAB
                # mode B runs pre-barrier (Pool idle: no gathers yet);
                # mode A runs inside conv2 where Pool is gather-bound
                eng = ((nc.sync, nc.gpsimd, nc.scalar)[i % 3]
                       if mode == "B" else (nc.sync, nc.scalar)[i % 2])
                xe = pool.tile([XROWS, 2 * MAC * SLAB], BF, tag="xe")
                if "c1load" not in sens:
                    eng.dma_start(
                        out=xe[0:xr, 0:2 * CW].rearrange(
                            "p (two n) -> p two n", two=2),
                        in_=src_d[0:xr, :, c0:c0 + CW])
                msgT = pool.tile([XROWS, MAC * SLAB], BF, tag="msgT")
                if "c1mul" not in sens:
                    anye.tensor_mul(out=msgT[0:xr, 0:CW], in0=xe[0:xr, 0:CW],
                                    in1=xe[0:xr, CW:2 * CW])
                if mode == "A":
                    for half in range(CW // 512):
                        ps = psC.tile([P, 512], FP, tag="pC")
                        if "c1mm" not in sens:
                            nc.tensor.matmul(
                                ps[:], lhsT=w1[0:xr, :],
                                rhs=msgT[0:xr, half * 512:(half + 1) * 512],
                                start=True, stop=True)
                        if "c1relu" not in sens:
                            anye.tensor_relu(
                                out=hresT[:, c0 + half * 512:c0 + (half + 1) * 512],
                                in_=ps[:])
                else:
                    h1m = pool.tile([P, MAC * GRP * F1], BF, tag="h1m")
                    for half in range(CW // 512):
                        ps = psC.tile([P, 512], FP, tag="pC")
                        for j in range(4) if "c1mm" not in sens else []:
                            jj = half * 4 + j
                            nc.tensor.matmul(
                                ps[:, j * P:(j + 1) * P],
                                lhsT=msgT[0:xr, jj * P:(jj + 1) * P],
                                rhs=w1[0:xr, :], start=True, stop=True)
                        if "c1relu" not in sens:
                            anye.tensor_relu(
                                out=h1m[:, half * 512:(half + 1) * 512],
                                in_=ps[:])
                    # one write: 128 descriptors x nb*256B contiguous
                    wr_eng = (nc.gpsimd, nc.scalar, nc.sync)[i % 3]
                    nb = CW // P
                    B0 = dst_rows // P
                    if "c1wr" not in sens:
                        wr_eng.dma_start(
                            out=tabAv[:, B0:B0 + nb, :],
                            in_=h1m[:, 0:nb * F1].rearrange(
                                "p (b f) -> p b f", f=F1))

            def macro_list(nsl_tot):
                out = []
                s = 0
                while s < nsl_tot:
                    out.append((s, min(MAC, nsl_tot - s)))
                    s += MAC
                return out

            mi = 0
            for (s0, nsl) in macro_list(NSL_SV):
                conv1_macro(xeSV_d, s0, nsl, XR_SV, mi, "B",
                            dst_rows=s0 * SLAB)
                mi += 1
            for (s0, nsl) in macro_list(NSL_H):
                conv1_macro(xeH_d, s0, nsl, XR_H, mi, "B",
                            dst_rows=n_SV_pad + s0 * SLAB)
                mi += 1

            tc.strict_bb_all_engine_barrier()
            # V macros only write SBUF (hresT) -- emitted after the barrier,
            # interleaved with conv2 groups that consume them
            vdone = 0
            vmi = 0
            if lvl < 1:
                for (s0, nsl) in macro_list(NSL_V):
                    conv1_macro(xeV_d, s0, nsl, XR_V, s0 // MAC, "A")
            if stop_after == "conv1bb":
                tc.strict_bb_all_engine_barrier()

            # ---------------- conv2 + pool ----------------
            if lvl >= 1:
                gat_t = cpool.tile([128, E2_slots // 16], mybir.dt.int16,
                                   tag="gat")
                nc.sync.dma_start(out=gat_t[:], in_=gat_d[:])
                s3all = cpool.tile([P, NT_V * MAXG_TILE], BF, tag="s3all")
                nc.scalar.dma_start(
                    out=s3all[:].rearrange("p (t g) -> p t g", g=MAXG_TILE),
                    in_=s3_d[:].rearrange("t p g -> p t g"))
                pgall = cpool.tile([P, NT_V // 4], mybir.dt.int32, tag="pgall")
                nc.sync.dma_start(
                    out=pgall[:].rearrange("p (b one) -> p b one", one=1),
                    in_=pgid_d[:].rearrange("b p one -> p b one"))
                sub_start = np.concatenate([[0], np.cumsum(T_sub)]).astype(int)
                plan = []
                t = 0
                while t < NT_V:
                    te = t
                    while te < NT_V and sub_start[te + 1] - sub_start[t] <= GG:
                        te += 1
                    plan.append((t, te))
                    t = te
                pool_ps = None
                tsrc = {}           # tile -> (gt, s2sl, so)
                assert NT_V % 2 == 0

                def emit_pair(t0):
                    # two tiles share one agg psum bank + one h2 psum bank,
                    # halving the PSUM->SBUF copy / relu op count
                    aggp = psA.tile([P, 2 * P], FP, tag="pA")
                    for q in range(2):
                        gt, s2sl, so = tsrc.pop(t0 + q)
                        s = 0
                        for w in range(NWIN):
                            nw_ = wsub[t0 + q][w]
                            for k in range(nw_):
                                nc.tensor.matmul(
                                    aggp[:, q * P + w * WW:
                                         q * P + (w + 1) * WW],
                                    lhsT=gt[:, so + s, :],
                                    rhs=s2sl[:, (so + s) * WW:
                                             (so + s + 1) * WW],
                                    start=(k == 0), stop=(k == nw_ - 1))
                                s += 1
                    aggs = pool.tile([P, 2 * P], BF, tag="agg2Ts")
                    anye.tensor_copy(out=aggs[:], in_=aggp[:])
                    h2p = psH.tile([P, 2 * F2], FP, tag="pB")
                    for q in range(2):
                        t = t0 + q
                        nc.tensor.matmul(h2p[:, q * F2:(q + 1) * F2],
                                         lhsT=aggs[:, q * P:(q + 1) * P],
                                         rhs=wrel2[:], start=True, stop=False)
                        nc.tensor.matmul(h2p[:, q * F2:(q + 1) * F2],
                                         lhsT=hresT[:, t * P:(t + 1) * P],
                                         rhs=wroot2[:], start=False, stop=True)
                    h2s = pool.tile([P, 2 * F2], BF, tag="h2s")
                    anye.tensor_relu(out=h2s[:], in_=h2p[:])
                    for q in range(2):
                        t = t0 + q
                        jj = t % 4
                        if jj == 0:
                            pp = psP.tile([P, F2], FP, tag="pP")
                            pool_ps[0] = pp
                        nc.tensor.matmul(
                            pool_ps[0][32 * jj:32 * jj + 32, :],
                            lhsT=s3all[:, t * MAXG_TILE:(t + 1) * MAXG_TILE],
                            rhs=h2s[:, q * F2:(q + 1) * F2],
                            start=True, stop=True, tile_position=(0, 32 * jj))
                        if jj == 3 or t == NT_V - 1:
                            npart = 32 * (jj + 1)
                            pls = pool.tile([P, F2], BF, tag="pls")
                            anye.tensor_copy(out=pls[:npart, :],
                                               in_=pool_ps[0][:npart, :])
                            nc.gpsimd.indirect_dma_start(
                                out=emb_d[:, :],
                                out_offset=bass.IndirectOffsetOnAxis(
                                    ap=pgall[:npart, t // 4:t // 4 + 1], axis=0),
                                in_=pls[:npart, :], in_offset=None,
                                bounds_check=GCOLS, oob_is_err=False)

                pool_ps = [None]
                for gi_, (ta, te) in enumerate(plan):
                    while vdone * GRP < te:
                        nsl = min(MAC, NSL_V - vdone)
                        conv1_macro(xeV_d, vdone, nsl, XR_V, vmi, "A")
                        vdone += nsl
                        vmi += 1
                    ns = int(sub_start[te] - sub_start[ta])
                    gt = bigp.tile([P, GG, F1], BF, tag="g2")
                    # SWDGE ring holds 1024 descs -> <=8 subtiles/gather
                    for q0 in range(0, ns, 8):
                        qn = min(8, ns - q0)
                        a8 = int(sub_start[ta]) + q0
                        nc.gpsimd.dma_gather(
                            gt[:, q0:q0 + qn, :], tabA_d[:],
                            gat_t[:, a8 * 8:(a8 + qn) * 8],
                            qn * P, qn * P, F1)
                    s2sl = bigp.tile([P, GG * WW], BF, tag="s2sl")
                    nc.sync.dma_start(
                        out=s2sl[:, :ns * WW],
                        in_=s2_d[:, int(sub_start[ta]) * WW:int(sub_start[te]) * WW])
                    for t in range(ta, te):
                        tsrc[t] = (gt, s2sl, int(sub_start[t] - sub_start[ta]))
                        if t % 2 == 1:
                            emit_pair(t - 1)

                while vdone < NSL_V:
                    nsl = min(MAC, NSL_V - vdone)
                    conv1_macro(xeV_d, vdone, nsl, XR_V, vmi, "A")
                    vdone += nsl
                    vmi += 1
                tc.strict_bb_all_engine_barrier()

            # ---------------- GRU (round-major emb pipeline) ----------------
            if lvl >= 2:
                gruw = cpool.tile([P, 15 * P], BF, tag="gruw")
                nc.sync.dma_start(
                    out=gruw[:].rearrange("p (w q) -> p w q", w=15),
                    in_=gruw_d[:].rearrange("w p q -> p w q"))
                dec = cpool.tile([P, 12], BF, tag="dec")
                nc.scalar.dma_start(out=dec[:], in_=dec_d[:])
                am = cpool.tile([12, NSH], FP, tag="am")
                nc.scalar.dma_start(out=am[:], in_=amask_d[:])

                # gruw cols: wih0 g0k0,g0k1,g1k0,g1k1,g2k0,g2k1 | whh0 x3
                #            | wih1 x3 | whh1 x3
                def gw(i):
                    return gruw[:, i * P:(i + 1) * P]

                wih0 = [gw(i) for i in range(6)]
                whh0 = [gw(6 + i) for i in range(3)]
                wih1 = [gw(9 + i) for i in range(3)]
                whh1 = [gw(12 + i) for i in range(3)]

                h0 = cpool.tile([P, NSH], BF, tag="h_L0")
                nc.gpsimd.memset(h0[:], 0.0)
                h1 = cpool.tile([P, NSH], BF, tag="h_L1")
                nc.gpsimd.memset(h1[:], 0.0)

                def gates_to_h(ps, gin_n, h, nm):
                    """ps cols [0:2N] hold r|z pre-activations (summed),
                    [2N:3N] gh_n; gin_n = gi n-gate AP. Updates h."""
                    rz = pool.tile([P, 2 * NSH], BF, tag=f"rz{nm}")
                    nc.scalar.activation(rz[:], ps[:, 0:2 * NSH], AF.Sigmoid)
                    ns_ = pool.tile([P, NSH], BF, tag=f"ns{nm}")
                    anye.tensor_mul(out=ns_[:], in0=rz[:, 0:NSH],
                                    in1=ps[:, 2 * NSH:3 * NSH])
                    anye.tensor_add(out=ns_[:], in0=ns_[:], in1=gin_n)
                    nc.scalar.activation(ns_[:], ns_[:], AF.Tanh)
                    hmn = pool.tile([P, NSH], BF, tag=f"hmn{nm}")
                    anye.tensor_sub(out=hmn[:], in0=h[:], in1=ns_[:])
                    anye.tensor_mul(out=hmn[:], in0=hmn[:],
                                    in1=rz[:, NSH:2 * NSH])
                    anye.tensor_add(out=h[:], in0=ns_[:], in1=hmn[:])

                for t in range(TR):
                    # load + transpose round-t embeddings: xt [feat-half, k, shot]
                    et = pool.tile([P, F2], BF, tag="et")
                    nc.sync.dma_start(out=et[:], in_=emb_d[t * P:(t + 1) * P, :])
                    xt = pool.tile([P, 2, P], BF, tag="xt")
                    for half in range(2):
                        tp = psA.tile([P, P], FP, tag="pA")
                        nc.tensor.matmul(tp[:], lhsT=et[:, half * P:(half + 1) * P],
                                         rhs=ident[:], start=True, stop=True)
                        anye.tensor_copy(out=xt[:, half, :], in_=tp[:])

                    # L0 step t: psum regions [r|z] = wih0@x_t + whh0@h0,
                    # [2N:3N] = whh0_n@h0, [3N:4N] = wih0_n@x_t
                    ps0 = psC.tile([P, 512], FP, tag="pC")
                    for gate in range(2):
                        reg = ps0[:, gate * NSH:(gate + 1) * NSH]
                        nc.tensor.matmul(reg, lhsT=whh0[gate], rhs=h0[:],
                                         start=True, stop=False)
                        nc.tensor.matmul(reg, lhsT=wih0[gate * 2], rhs=xt[:, 0, :],
                                         start=False, stop=False)
                        nc.tensor.matmul(reg, lhsT=wih0[gate * 2 + 1],
                                         rhs=xt[:, 1, :], start=False, stop=True)
                    nc.tensor.matmul(ps0[:, 2 * NSH:3 * NSH], lhsT=whh0[2],
                                     rhs=h0[:], start=True, stop=True)
                    nc.tensor.matmul(ps0[:, 3 * NSH:4 * NSH], lhsT=wih0[4],
                                     rhs=xt[:, 0, :], start=True, stop=False)
                    nc.tensor.matmul(ps0[:, 3 * NSH:4 * NSH], lhsT=wih0[5],
                                     rhs=xt[:, 1, :], start=False, stop=True)
                    gates_to_h(ps0, ps0[:, 3 * NSH:4 * NSH], h0, "0")

                    # L1 step t
                    ps1 = psC.tile([P, 512], FP, tag="pC")
                    for gate in range(2):
                        reg = ps1[:, gate * NSH:(gate + 1) * NSH]
                        nc.tensor.matmul(reg, lhsT=wih1[gate], rhs=h0[:],
                                         start=True, stop=False)
                        nc.tensor.matmul(reg, lhsT=whh1[gate], rhs=h1[:],
                                         start=False, stop=True)
                    nc.tensor.matmul(ps1[:, 2 * NSH:3 * NSH], lhsT=whh1[2],
                                     rhs=h1[:], start=True, stop=True)
                    nc.tensor.matmul(ps1[:, 3 * NSH:4 * NSH], lhsT=wih1[2],
                                     rhs=h0[:], start=True, stop=True)
                    gates_to_h(ps1, ps1[:, 3 * NSH:4 * NSH], h1, "1")

                hlast = h1
                lp = psA.tile([P, P], FP, tag="pA")
                nc.tensor.matmul(lp[:12, :NSH], lhsT=dec[:], rhs=hlast[:],
                                 start=True, stop=True)
                lo = pool.tile([12, NSH], FP, tag="lo")
                nc.vector.tensor_mul(out=lo[:], in0=lp[:12, :NSH], in1=am[:])
                nc.sync.dma_start(out=out_d[:], in_=lo[:])

            else:
                lo = pool.tile([12, NSH], FP, tag="lo")
                nc.gpsimd.memset(lo[:], 0.0)
                nc.sync.dma_start(out=out_d[:], in_=lo[:])

    nc.compile()
    return nc


def make_in_map(c, meta, W):
    """Per-core input arrays for run_bass_kernel_spmd."""
    A = c["arrays"]
    bf = lambda a: np.ascontiguousarray(a, dtype=BF16)
    f32 = lambda a: np.ascontiguousarray(a, dtype=np.float32)

    # w1full rows: slot*5+f -> wrel[f]; 80+f -> wroot[f]
    w1 = np.zeros((P, P), np.float32)
    w1[:F_IN] = f32(W["c1_wroot"])
    w1[F_IN:XROWS] = np.tile(f32(W["c1_wrel"]), (KSLOT, 1))
    wih0 = [f32(W["w_ih0"])[g * P:(g + 1) * P, k * P:(k + 1) * P].T
            for g in range(3) for k in range(2)]
    whh0 = [f32(W["w_hh0"])[g * P:(g + 1) * P, :].T for g in range(3)]
    wih1 = [f32(W["w_ih1"])[g * P:(g + 1) * P, :].T for g in range(3)]
    whh1 = [f32(W["w_hh1"])[g * P:(g + 1) * P, :].T for g in range(3)]
    gruw = np.stack(wih0 + whh0 + wih1 + whh1)
    amask = np.broadcast_to(A["amask"][None, :], (12, meta["n_shot_core"]))

    return {
        "xeV": bf(A["xeV"]),
        "xeSV": bf(A["xeSV"]),
        "xeH": bf(A["xeH"]),
        "gat": np.ascontiguousarray(wrap_idx16(A["gat_idx"], meta["E2_slots"])),
        "s2": bf(A["s2f"]),
        "s3": bf(A["s3"]),
        "pgid": np.ascontiguousarray(
            A["pool_gid"].reshape(-1, P, 1), dtype=np.int32),
        "amask": f32(amask),
        "ident": bf(np.eye(P, dtype=np.float32)),
        "w1": bf(w1),
        "wrel2": bf(W["c2_wrel"]),
        "wroot2": bf(W["c2_wroot"]),
        "gruw": bf(gruw),
        "dec": bf(W["dec_w"]),
    }


# ------------------------------------------------------------------
_CACHE = {}


def _get_nc(meta):
    key = (meta["NT_V"], meta["NT_H"], meta["n_SV_pad"], meta["E2_slots"],
           meta["G_rows"], tuple(meta["T_sub"]),
           tuple(tuple(r) for r in meta["wsub"]),
           tuple(meta["XR_V"]), tuple(meta["XR_SV"]), tuple(meta["XR_H"]))
    if key not in _CACHE:
        _CACHE[key] = build(meta, num_devices=NC)
    return _CACHE[key]


def kernel(**inputs):
    import sys as _sys
    if "/opt/trn_rl_repo" not in _sys.path:
        _sys.path.insert(0, "/opt/trn_rl_repo")
    from concourse.bass_utils import run_bass_kernel_spmd

    for k in ("c1_b", "c2_b", "b_ih0", "b_hh0", "b_ih1", "b_hh1", "dec_b",
              "empty_emb"):
        assert not np.any(np.asarray(inputs[k])), f"nonzero {k} unsupported"

    cores, meta = prep(inputs)
    W = {k: np.asarray(v, np.float32) for k, v in inputs.items()
         if k not in ("x", "edge_index", "edge_attr", "batch_labels",
                      "label_map", "B")}
    nc = _get_nc(meta)
    in_maps = [make_in_map(c, meta, W) for c in cores]
    res = None
    for attempt in range(4):
        try:
            res = run_bass_kernel_spmd(nc, in_maps, core_ids=list(range(NC)))
            break
        except Exception:
            if attempt == 3:
                raise
    B = meta["B"]
    out = np.zeros((B, 12), np.float32)
    nsh = meta["n_shot_core"]
    for d in range(NC):
        lg = res.results[d]["out"]          # [12, nsh]
        s = d + NC * np.arange(nsh)
        out[s[s < B]] = lg.T[s < B]
    return out



# revision 20
# speedup vs baseline: 1.1160x; 1.0128x over previous
"""Host-side sharding/prep + Bass device program for nn_BBGRUDecoder.

Host code does index manipulation / data layout only (no model FLOPs);
the device kernel does all arithmetic.

Device-side structure (per core, SPMD over 8 cores; shots sharded
round-robin by shot % 8):

  conv1: node features staged transposed as [85, U_pad] bf16 where rows
    0..79 are the 16 in-edge slots x 5 features and rows 80..84 are the
    node's own features (eav rows are edge weights, 1.0 for the root
    block). msg = xs*eav elementwise, then ONE matmul per tile against
    w1full [85,128] (wrel tiled 16x + wroot) folds the slot reduction,
    the root term and the F1 projection. V tiles produce hresT [F1,node]
    (SBUF-resident, feeds conv2's root term); SV + halo tiles produce
    [node,F1] rows written to the DRAM gather table tabA.
  conv2: per 128-edge subtile, gather source rows from tabA (Pool SWDGE)
    and matmul against the sparse scatter matrix s2 (flat [128,E2_slots]
    layout for full-rate slab loads). Root term from hresT. Pool by
    graph via s3 0/1 matrices, scatter per-graph sums to emb (indirect
    DMA, graph-column ids).
  GRU: emb -> embT [128, GCOLS], batched input projections, 10-step
    recurrence x 2 layers, decoder matmul, active-shot mask.
"""
import numpy as np
import ml_dtypes

BF16 = np.dtype(ml_dtypes.bfloat16)
NC = 8
P = 128
KSLOT = 16       # conv1 in-edge slots per node (max in-degree 13)
F_IN = 5
F1 = 128
F2 = 256
HID = 128
TR = 10          # rounds per shot
MAXG_TILE = 32   # max graphs per node-tile (pool S3 width)
IDX_CAP = 32767
XROWS = KSLOT * F_IN + F_IN   # 85: root rows (first 5) + slot rows
GRP = 8          # node tiles per conv1 slab
GG = 16          # conv2 edge subtiles per gather group
WW = 32          # conv2 dst-window width (s2 block cols)
NWIN = P // WW   # dst windows per node tile


def _pack_groups(sizes, cap_items, cap_groups):
    """Greedy-pack consecutive groups (each <=cap_items items) into tiles of
    <=cap_items items and <=cap_groups groups. Returns list of (start_group,
    n_groups, n_items)."""
    tiles = []
    i = 0
    n = len(sizes)
    while i < n:
        items = 0
        g = 0
        while i + g < n and g < cap_groups and items + sizes[i + g] <= cap_items:
            items += sizes[i + g]
            g += 1
        assert g > 0, f"group {i} size {sizes[i]} exceeds cap {cap_items}"
        tiles.append((i, g, items))
        i += g
    return tiles


def prep(inputs):
    x = np.asarray(inputs["x"], np.float32)
    ei = np.asarray(inputs["edge_index"], np.int64)
    ea = np.asarray(inputs["edge_attr"], np.float32)
    bl = np.asarray(inputs["batch_labels"], np.int64)
    lm = np.asarray(inputs["label_map"], np.int64)
    B = int(inputs["B"])
    NN = x.shape[0]
    src_g, dst_g = ei[0], ei[1]
    shot_of, round_of = lm[:, 0], lm[:, 1]
    n_shot_core = (B + NC - 1) // NC          # 128 shots per core
    GCOLS = n_shot_core * TR                  # 1280 graph-columns per core
    deg = np.bincount(dst_g, minlength=NN)
    assert deg.max() <= KSLOT

    node_g = bl
    node_core = (shot_of[node_g] % NC).astype(np.int64)

    cores = []
    for d in range(NC):
        V = np.nonzero(node_core == d)[0]          # ascending node ids
        gids, gstart, gcnt = np.unique(node_g[V], return_index=True, return_counts=True)
        # order graphs by max node in-degree so slab XR (=5+5*maxdeg) shrinks
        gmax = np.maximum.reduceat(deg[V], gstart)
        gord = np.argsort(gmax, kind="stable")
        gids, gstart, gcnt = gids[gord], gstart[gord], gcnt[gord]
        # graph column within core: s_idx*TR + round
        s_idx = shot_of[gids] // NC
        # round-major: emb tile t holds exactly the round-t embeddings,
        # so the GRU consumes emb tiles in step order (pipelines with loads)
        gcol = round_of[gids] * n_shot_core + s_idx
        # ---- graph-aligned node tiles ----
        tiles = _pack_groups(gcnt.tolist(), P, MAXG_TILE)
        NT_V = len(tiles)
        vpos = np.full(NN, -1, np.int64)        # global node id -> packed pos
        packed_rows = []                        # per tile: global node ids (len<=P)
        tile_graphs = []                        # per tile: (gcol list, counts)
        for (g0, ng, ni) in tiles:
            rows = [V[gstart[k]:gstart[k] + gcnt[k]] for k in range(g0, g0 + ng)]
            rows = np.concatenate(rows)
            packed_rows.append(rows)
            tile_graphs.append((gcol[g0:g0 + ng], gcnt[g0:g0 + ng]))
        for t, rows in enumerate(packed_rows):
            vpos[rows] = t * P + np.arange(len(rows))

        # ---- conv2 edges (dst in V) ----
        E = np.nonzero(node_core[dst_g] == d)[0]
        e_src, e_dst, e_ea = src_g[E], dst_g[E], ea[E]
        S = np.unique(e_src)
        SV = S[node_core[S] == d]
        H = S[node_core[S] != d]
        H = H[np.argsort(deg[H], kind="stable")]   # degree-sorted slabs
        # table A (gather source) layout: [SV (degree-sorted), H]
        SVo = SV[np.argsort(deg[SV], kind="stable")]
        apos = np.full(NN, -1, np.int64)
        apos[SVo] = np.arange(len(SVo))
        cores.append(dict(
            d=d, V=V, NT_V=NT_V, tiles=tiles, packed_rows=packed_rows,
            tile_graphs=tile_graphs, vpos=vpos,
            E=E, e_src=e_src, e_dst=e_dst, e_ea=e_ea,
            SVo=SVo, H=H, apos=apos, gids=gids, gcol=gcol, gcnt=gcnt,
        ))

    # ---- shared static shapes ----
    NT_V = max(c["NT_V"] for c in cores)
    NT_V = -(-NT_V // GRP) * GRP              # pad to slab multiple
    n_SV_pad = -(-max(len(c["SVo"]) for c in cores) // (GRP * P)) * (GRP * P)
    NT_H = -(-max(len(c["H"]) for c in cores) // P)
    NT_H = -(-NT_H // GRP) * GRP
    H_pad = NT_H * P
    A_rows = n_SV_pad + H_pad
    assert A_rows <= IDX_CAP + 1, A_rows
    V_pad = NT_V * P

    # conv2: per-tile x per-dst-window edge subtile counts (max over cores).
    # Each window covers WW consecutive dst slots; a subtile is <=128 edges
    # whose dsts fall in that window -> s2 block is [128, WW] not [128, 128].
    wsub = np.ones((NT_V, NWIN), np.int64)
    for c in cores:
        vp = c["vpos"][c["e_dst"]]
        cnt = np.bincount(vp // WW, minlength=NT_V * NWIN)[:NT_V * NWIN]
        ns = np.maximum(-(-cnt // P), 1).reshape(-1, NWIN)
        n_t = ns.shape[0]
        wsub[:n_t] = np.maximum(wsub[:n_t], ns)
    T_sub = wsub.sum(axis=1)
    n_slots = int(T_sub.sum())
    E2_slots = n_slots * P

    # per-slab conv1 staging rows: XR = 5 (root) + 5*max_indegree_in_slab
    NSL_V_ = NT_V // GRP
    NSL_SV_ = n_SV_pad // (GRP * P)
    NSL_H_ = NT_H // GRP
    KV = np.zeros(NSL_V_, np.int64)
    KSV = np.zeros(NSL_SV_, np.int64)
    KH = np.zeros(NSL_H_, np.int64)
    for c in cores:
        for s in range(NSL_V_):
            rows = [c["packed_rows"][t]
                    for t in range(s * GRP, min((s + 1) * GRP, c["NT_V"]))]
            rows = np.concatenate(rows) if rows else np.zeros(0, np.int64)
            if len(rows):
                KV[s] = max(KV[s], deg[rows].max())
        for s in range(NSL_SV_):
            rows = c["SVo"][s * GRP * P:(s + 1) * GRP * P]
            if len(rows):
                KSV[s] = max(KSV[s], deg[rows].max())
        for s in range(NSL_H_):
            rows = c["H"][s * GRP * P:(s + 1) * GRP * P]
            if len(rows):
                KH[s] = max(KH[s], deg[rows].max())
    XR_V = (F_IN + F_IN * KV).tolist()
    XR_SV = (F_IN + F_IN * KSV).tolist()
    XR_H = (F_IN + F_IN * KH).tolist()

    meta = dict(NT_V=NT_V, NT_H=NT_H, V_pad=V_pad, H_pad=H_pad,
                n_SV_pad=n_SV_pad, A_rows=A_rows, T_sub=T_sub.tolist(),
                wsub=wsub.tolist(), XR_V=XR_V, XR_SV=XR_SV, XR_H=XR_H,
                E2_slots=E2_slots, GCOLS=GCOLS, G_rows=-(-(GCOLS + 1) // P) * P,
                n_shot_core=n_shot_core, B=B)

    # ---- per-core padded arrays ----
    ones5 = np.ones(F_IN, np.float32)
    for c in cores:
        vpos, apos = c["vpos"], c["apos"]

        def slot_arrays(nodes, npad):
            """Transposed conv1 staging [XROWS, 2, npad] for the given
            node list (bf16): [:,0]=feature values, [:,1]=edge weights."""
            xs = np.zeros((npad, KSLOT, F_IN), np.float32)
            ev = np.zeros((npad, KSLOT, F_IN), np.float32)
            pos = np.full(NN, -1, np.int64)
            pos[nodes] = np.arange(len(nodes))
            # in-edges of these nodes from FULL edge list
            EU = np.nonzero(pos[dst_g] >= 0)[0]
            du = pos[dst_g[EU]]
            order = np.argsort(du, kind="stable")
            EU, du = EU[order], du[order]
            # vectorized running slot index within each dst group
            uniq, first = np.unique(du, return_index=True)
            sl = np.arange(len(EU)) - np.repeat(first, np.diff(
                np.concatenate([first, [len(EU)]])))
            xs[du, sl] = x[src_g[EU]]
            ev[du, sl] = ea[EU][:, None] * ones5
            out = np.zeros((XROWS, 2, npad), np.float32)
            out[:F_IN, 0, :len(nodes)] = x[nodes].T
            out[:F_IN, 1, :len(nodes)] = 1.0
            out[F_IN:, 0] = xs.reshape(npad, -1).T
            out[F_IN:, 1] = ev.reshape(npad, -1).T
            return out.astype(BF16)

        U_nodes = np.full(V_pad, -1, np.int64)
        for t, rows in enumerate(c["packed_rows"]):
            U_nodes[t * P:t * P + len(rows)] = rows
        # U_nodes has -1 gaps between tiles; compute compact, scatter to packed
        xeV = np.zeros((XROWS, 2, V_pad), BF16)
        m = U_nodes >= 0
        xeV[:, :, np.nonzero(m)[0]] = slot_arrays(U_nodes[m], int(m.sum()))
        xeSV = slot_arrays(c["SVo"], n_SV_pad)
        xeH = slot_arrays(c["H"], H_pad)

        # conv2 gather idx + s2 flat [P, n_slots*WW], per (tile, dst-window)
        gat_idx = np.zeros(E2_slots, np.int64)
        s2f = np.zeros((P, n_slots * WW), np.float32)
        hpos = np.full(NN, -1, np.int64)
        hpos[c["H"]] = np.arange(len(c["H"]))
        vp = vpos[c["e_dst"]]
        eord = np.argsort(vp, kind="stable")
        bnd = np.searchsorted(vp[eord], np.arange(0, NT_V * P + 1, WW))
        st = 0
        for t in range(NT_V):
            for w in range(NWIN):
                nsub = int(wsub[t, w])
                if t < c["NT_V"]:
                    k = t * NWIN + w
                    sel = eord[bnd[k]:bnd[k + 1]]
                    es, ed, ew = c["e_src"][sel], c["e_dst"][sel], c["e_ea"][sel]
                    # gather position in table A: SV->apos, H->n_SV_pad+pos
                    ga = np.where(apos[es] >= 0, apos[es], n_SV_pad + hpos[es])
                    ne = len(es)
                    if ne:
                        assert (ga >= 0).all() and (ga < A_rows).all()
                        assert ne <= nsub * P
                        i = np.arange(ne)
                        # tabA is partition-major: packed pos n lives at
                        # virtual row (n%P)*NTB + n//P (one 8KB descriptor
                        # per partition on the batched conv1 writes)
                        gat_idx[st * P + i] = (ga % P) * (A_rows // P) + ga // P
                        s2f[i % P, (st + i // P) * WW
                            + (vpos[ed] - t * P - w * WW)] = ew
                st += nsub
        assert st == n_slots

        # pool S3 [NT_V, P, MAXG_TILE] 0/1, graph ids / inv counts
        s3 = np.zeros((NT_V, P, MAXG_TILE), np.float32)
        pool_gid = np.full((NT_V, MAXG_TILE), meta["GCOLS"] + 100, np.int64)
        for t in range(c["NT_V"]):
            gcols, gcnts = c["tile_graphs"][t]
            off = 0
            for j, (gc, n) in enumerate(zip(gcols, gcnts)):
                s3[t, off:off + n, j] = 1.0 / n
                pool_gid[t, j] = gc
                off += n

        # active-shot mask for decoder
        amask = np.zeros(meta["n_shot_core"], np.float32)
        amask[(shot_of[c["gids"]] // NC)] = 1.0

        c["arrays"] = dict(
            xeV=xeV, xeSV=xeSV, xeH=xeH, gat_idx=gat_idx, s2f=s2f.astype(BF16),
            s3=s3.astype(BF16), pool_gid=pool_gid, amask=amask,
        )
    return cores, meta


def wrap_idx16(idx, pad_to):
    """int idx array -> dma_gather int16 layout [128, pad_to//16], 0 padded."""
    a = np.zeros(pad_to, np.int16)
    a[:len(idx)] = idx.astype(np.int16)
    w = a.reshape(pad_to // 16, 16).T  # [16, C]
    return np.tile(w, (8, 1)).copy()


# ======================================================
import sys as _sys
if "/opt/trn_rl_repo" not in _sys.path:
    _sys.path.insert(0, "/opt/trn_rl_repo")
import concourse.bass as bass
import concourse.bacc as bacc
import concourse.mybir as mybir
from concourse.tile import TileContext


BF = mybir.dt.bfloat16
FP = mybir.dt.float32
AF = mybir.ActivationFunctionType


def build(meta, num_devices=8, stop_after="full", use_any=True, c2sub="all", sens=()):
    NT_V, NT_H = meta["NT_V"], meta["NT_H"]
    V_pad, H_pad = meta["V_pad"], meta["H_pad"]
    n_SV_pad, A_rows = meta["n_SV_pad"], meta["A_rows"]
    T_sub = meta["T_sub"]
    wsub = meta["wsub"]
    XR_V, XR_SV, XR_H = meta["XR_V"], meta["XR_SV"], meta["XR_H"]
    E2_slots = meta["E2_slots"]
    n_slots = E2_slots // P
    GCOLS, G_rows = meta["GCOLS"], meta["G_rows"]
    NSH = meta["n_shot_core"]
    NT_G = G_rows // P

    nc = bacc.Bacc("TRN2", target_bir_lowering=False, debug=False,
                   num_devices=num_devices)

    def inp(name, shape, dt):
        return nc.dram_tensor(name, shape, dt, kind="ExternalInput")

    xeV_d = inp("xeV", [XROWS, 2, V_pad], BF)
    xeSV_d = inp("xeSV", [XROWS, 2, n_SV_pad], BF)
    xeH_d = inp("xeH", [XROWS, 2, H_pad], BF)
    gat_d = inp("gat", [128, E2_slots // 16], mybir.dt.int16)
    s2_d = inp("s2", [P, n_slots * WW], BF)
    s3_d = inp("s3", [NT_V, P, MAXG_TILE], BF)
    pgid_d = inp("pgid", [NT_V // 4, P, 1], mybir.dt.int32)
    amask_d = inp("amask", [12, NSH], FP)
    ident_d = inp("ident", [P, P], BF)
    w1_d = inp("w1", [P, P], BF)           # rows 0..84 = w1full
    wrel2_d = inp("wrel2", [P, F2], BF)
    wroot2_d = inp("wroot2", [P, F2], BF)
    gruw_d = inp("gruw", [15, P, P], BF)   # wih0(6), whh0(3), wih1(3), whh1(3)
    dec_d = inp("dec", [P, 12], BF)
    out_d = nc.dram_tensor("out", [12, NSH], FP, kind="ExternalOutput")

    tabA_d = nc.dram_tensor("tabA", [A_rows, F1], BF, kind="Internal")
    emb_d = nc.dram_tensor("emb", [G_rows, F2], BF, kind="Internal")

    lvl = ("conv1", "conv1bb", "conv2", "full").index(stop_after) - 1

    NSL_V = NT_V // GRP
    NSL_SV = n_SV_pad // (GRP * P)
    NSL_H = NT_H // GRP
    SLAB = GRP * P   # 1024 columns per slab

    with TileContext(nc) as tc:
        with (
            tc.tile_pool(name="const", bufs=1) as cpool,
            tc.tile_pool(name="sb", bufs=3) as pool,
            tc.tile_pool(name="big", bufs=3) as bigp,
            tc.tile_pool(name="gi", bufs=1) as gip,
            tc.tile_pool(name="psA", bufs=2, space="PSUM") as psA,
            tc.tile_pool(name="psH", bufs=2, space="PSUM") as psH,
            tc.tile_pool(name="psP", bufs=1, space="PSUM") as psP,
            tc.tile_pool(name="psC", bufs=3, space="PSUM") as psC,
        ):
            anye = nc.any if use_any else nc.vector
            ident = cpool.tile([P, P], BF, tag="ident")
            nc.sync.dma_start(out=ident[:], in_=ident_d[:])
            w1 = cpool.tile([P, P], BF, tag="w1")
            nc.sync.dma_start(out=w1[:], in_=w1_d[:])
            wrel2 = cpool.tile([P, F2], BF, tag="wrel2")
            nc.scalar.dma_start(out=wrel2[:], in_=wrel2_d[:])
            wroot2 = cpool.tile([P, F2], BF, tag="wroot2")
            nc.scalar.dma_start(out=wroot2[:], in_=wroot2_d[:])
            # hresT: conv1 output [F1, node] for V tiles, SBUF-resident
            hresT = cpool.tile([P, V_pad], BF, tag="hresT")

            zt = cpool.tile([P, NT_G * F2], BF, tag="zero")
            nc.gpsimd.memset(zt[:], 0.0)
            nc.sync.dma_start(
                out=emb_d[:].rearrange("(b p) f -> p b f", p=P),
                in_=zt[:].rearrange("p (b f) -> p b f", f=F2))

            # ---------------- conv1 ----------------
            # MAC slabs per macro: one DMA load (and one tabA write) covers
            # MAC slabs -- DMA queue time is fixed-overhead dominated
            # (~1.5us/instruction), so batch aggressively.
            MAC = 4
            tabAv = tabA_d[:].rearrange("(p b) f -> p b f", p=P)

            def conv1_macro(src_d, s0, nsl, XRs, i, mode, dst_rows=None):
                """nsl slabs starting at slab s0: load, msg-mul, per-slab
                matmul+relu. mode 'A': -> hresT cols; 'B': -> tabA rows
                (partition-major virtual layout) at dst_rows."""
                xr = max(XRs[s0:s0 + nsl])
                c0 = s0 * SLAB
                CW = nsl * SLAB
                # mode B runs pre-barrier (Pool idle: no gathers yet);
                # mode A runs inside conv2 where Pool is gather-bound
                eng = ((nc.sync, nc.gpsimd)[i % 2]
                       if mode == "B" else nc.sync)
                xe = pool.tile([XROWS, 2 * MAC * SLAB], BF, tag="xe")
                if "c1load" not in sens:
                    eng.dma_start(
                        out=xe[0:xr, 0:2 * CW].rearrange(
                            "p (two n) -> p two n", two=2),
                        in_=src_d[0:xr, :, c0:c0 + CW])
                msgT = pool.tile([XROWS, MAC * SLAB], BF, tag="msgT")
                if "c1mul" not in sens:
                    anye.tensor_mul(out=msgT[0:xr, 0:CW], in0=xe[0:xr, 0:CW],
                                    in1=xe[0:xr, CW:2 * CW])
                if mode == "A":
                    for half in range(CW // 512):
                        ps = psC.tile([P, 512], FP, tag="pC")
                        if "c1mm" not in sens:
                            nc.tensor.matmul(
                                ps[:], lhsT=w1[0:xr, :],
                                rhs=msgT[0:xr, half * 512:(half + 1) * 512],
                                start=True, stop=True)
                        if "c1relu" not in sens:
                            anye.tensor_relu(
                                out=hresT[:, c0 + half * 512:c0 + (half + 1) * 512],
                                in_=ps[:])
                else:
                    h1m = pool.tile([P, MAC * GRP * F1], BF, tag="h1m")
                    for half in range(CW // 512):
                        ps = psC.tile([P, 512], FP, tag="pC")
                        for j in range(4) if "c1mm" not in sens else []:
                            jj = half * 4 + j
                            nc.tensor.matmul(
                                ps[:, j * P:(j + 1) * P],
                                lhsT=msgT[0:xr, jj * P:(jj + 1) * P],
                                rhs=w1[0:xr, :], start=True, stop=True)
                        if "c1relu" not in sens:
                            anye.tensor_relu(
                                out=h1m[:, half * 512:(half + 1) * 512],
                                in_=ps[:])
                    # one write: 128 descriptors x nb*256B contiguous
                    wr_eng = (nc.gpsimd, nc.scalar, nc.sync)[i % 3]
                    nb = CW // P
                    B0 = dst_rows // P
                    if "c1wr" not in sens:
                        wr_eng.dma_start(
                            out=tabAv[:, B0:B0 + nb, :],
                            in_=h1m[:, 0:nb * F1].rearrange(
                                "p (b f) -> p b f", f=F1))

            def macro_list(nsl_tot, XRs):
                # group consecutive slabs only while XR stays close, so one
                # high-degree tail slab doesn't inflate the whole macro load
                out = []
                s = 0
                while s < nsl_tot:
                    n = 1
                    while (n < MAC and s + n < nsl_tot
                           and XRs[s + n] - XRs[s] <= 10):
                        n += 1
                    out.append((s, n))
                    s += n
                return out

            mi = 0
            for (s0, nsl) in macro_list(NSL_SV, XR_SV):
                conv1_macro(xeSV_d, s0, nsl, XR_SV, mi, "B",
                            dst_rows=s0 * SLAB)
                mi += 1
            for (s0, nsl) in macro_list(NSL_H, XR_H):
                conv1_macro(xeH_d, s0, nsl, XR_H, mi, "B",
                            dst_rows=n_SV_pad + s0 * SLAB)
                mi += 1

            tc.strict_bb_all_engine_barrier()
            # V macros only write SBUF (hresT) -- emitted after the barrier,
            # interleaved with conv2 groups that consume them
            vdone = 0
            vmi = 0
            if lvl < 1:
                for (s0, nsl) in macro_list(NSL_V, XR_V):
                    conv1_macro(xeV_d, s0, nsl, XR_V, s0 // MAC, "A")
            if stop_after == "conv1bb":
                tc.strict_bb_all_engine_barrier()

            # ---------------- conv2 + pool ----------------
            if lvl >= 1:
                gat_t = cpool.tile([128, E2_slots // 16], mybir.dt.int16,
                                   tag="gat")
                nc.sync.dma_start(out=gat_t[:], in_=gat_d[:])
                s3all = cpool.tile([P, NT_V * MAXG_TILE], BF, tag="s3all")
                nc.scalar.dma_start(
                    out=s3all[:].rearrange("p (t g) -> p t g", g=MAXG_TILE),
                    in_=s3_d[:].rearrange("t p g -> p t g"))
                pgall = cpool.tile([P, NT_V // 4], mybir.dt.int32, tag="pgall")
                nc.sync.dma_start(
                    out=pgall[:].rearrange("p (b one) -> p b one", one=1),
                    in_=pgid_d[:].rearrange("b p one -> p b one"))
                sub_start = np.concatenate([[0], np.cumsum(T_sub)]).astype(int)
                plan = []
                t = 0
                while t < NT_V:
                    te = t
                    while te < NT_V and sub_start[te + 1] - sub_start[t] <= GG:
                        te += 1
                    plan.append((t, te))
                    t = te
                pool_ps = None
                tsrc = {}           # tile -> (gt, s2sl, so)
                assert NT_V % 2 == 0

                def emit_pair(t0):
                    # two tiles share one agg psum bank + one h2 psum bank,
                    # halving the PSUM->SBUF copy / relu op count
                    aggp = psA.tile([P, 2 * P], FP, tag="pA")
                    for q in range(2):
                        gt, s2sl, so = tsrc.pop(t0 + q)
                        s = 0
                        for w in range(NWIN):
                            nw_ = wsub[t0 + q][w]
                            for k in range(nw_):
                                nc.tensor.matmul(
                                    aggp[:, q * P + w * WW:
                                         q * P + (w + 1) * WW],
                                    lhsT=gt[:, so + s, :],
                                    rhs=s2sl[:, (so + s) * WW:
                                             (so + s + 1) * WW],
                                    start=(k == 0), stop=(k == nw_ - 1))
                                s += 1
                    aggs = pool.tile([P, 2 * P], BF, tag="agg2Ts")
                    anye.tensor_copy(out=aggs[:], in_=aggp[:])
                    h2p = psH.tile([P, 2 * F2], FP, tag="pB")
                    for q in range(2):
                        t = t0 + q
                        nc.tensor.matmul(h2p[:, q * F2:(q + 1) * F2],
                                         lhsT=aggs[:, q * P:(q + 1) * P],
                                         rhs=wrel2[:], start=True, stop=False)
                        nc.tensor.matmul(h2p[:, q * F2:(q + 1) * F2],
                                         lhsT=hresT[:, t * P:(t + 1) * P],
                                         rhs=wroot2[:], start=False, stop=True)
                    h2s = pool.tile([P, 2 * F2], BF, tag="h2s")
                    anye.tensor_relu(out=h2s[:], in_=h2p[:])
                    for q in range(2):
                        t = t0 + q
                        jj = t % 4
                        if jj == 0:
                            pp = psP.tile([P, F2], FP, tag="pP")
                            pool_ps[0] = pp
                        nc.tensor.matmul(
                            pool_ps[0][32 * jj:32 * jj + 32, :],
                            lhsT=s3all[:, t * MAXG_TILE:(t + 1) * MAXG_TILE],
                            rhs=h2s[:, q * F2:(q + 1) * F2],
                            start=True, stop=True, tile_position=(0, 32 * jj))
                        if jj == 3 or t == NT_V - 1:
                            npart = 32 * (jj + 1)
                            pls = pool.tile([P, F2], BF, tag="pls")
                            anye.tensor_copy(out=pls[:npart, :],
                                               in_=pool_ps[0][:npart, :])
                            nc.gpsimd.indirect_dma_start(
                                out=emb_d[:, :],
                                out_offset=bass.IndirectOffsetOnAxis(
                                    ap=pgall[:npart, t // 4:t // 4 + 1], axis=0),
                                in_=pls[:npart, :], in_offset=None,
                                bounds_check=GCOLS, oob_is_err=False)

                pool_ps = [None]
                vmacros = macro_list(NSL_V, XR_V)
                for gi_, (ta, te) in enumerate(plan):
                    while vdone * GRP < te:
                        s0, nsl = vmacros[vmi]
                        conv1_macro(xeV_d, s0, nsl, XR_V, vmi, "A")
                        vdone += nsl
                        vmi += 1
                    ns = int(sub_start[te] - sub_start[ta])
                    gt = bigp.tile([P, GG, F1], BF, tag="g2")
                    # SWDGE ring holds 1024 descs -> <=8 subtiles/gather
                    for q0 in range(0, ns, 8):
                        qn = min(8, ns - q0)
                        a8 = int(sub_start[ta]) + q0
                        nc.gpsimd.dma_gather(
                            gt[:, q0:q0 + qn, :], tabA_d[:],
                            gat_t[:, a8 * 8:(a8 + qn) * 8],
                            qn * P, qn * P, F1)
                    s2sl = bigp.tile([P, GG * WW], BF, tag="s2sl")
                    nc.sync.dma_start(
                        out=s2sl[:, :ns * WW],
                        in_=s2_d[:, int(sub_start[ta]) * WW:int(sub_start[te]) * WW])
                    for t in range(ta, te):
                        tsrc[t] = (gt, s2sl, int(sub_start[t] - sub_start[ta]))
                        if t % 2 == 1:
                            emit_pair(t - 1)

                while vdone < NSL_V:
                    s0, nsl = vmacros[vmi]
                    conv1_macro(xeV_d, s0, nsl, XR_V, vmi, "A")
                    vdone += nsl
                    vmi += 1
                tc.strict_bb_all_engine_barrier()

            # ---------------- GRU (round-major emb pipeline) ----------------
            if lvl >= 2:
                gruw = cpool.tile([P, 15 * P], BF, tag="gruw")
                nc.sync.dma_start(
                    out=gruw[:].rearrange("p (w q) -> p w q", w=15),
                    in_=gruw_d[:].rearrange("w p q -> p w q"))
                dec = cpool.tile([P, 12], BF, tag="dec")
                nc.scalar.dma_start(out=dec[:], in_=dec_d[:])
                am = cpool.tile([12, NSH], FP, tag="am")
                nc.scalar.dma_start(out=am[:], in_=amask_d[:])

                # gruw cols: wih0 g0k0,g0k1,g1k0,g1k1,g2k0,g2k1 | whh0 x3
                #            | wih1 x3 | whh1 x3
                def gw(i):
                    return gruw[:, i * P:(i + 1) * P]

                wih0 = [gw(i) for i in range(6)]
                whh0 = [gw(6 + i) for i in range(3)]
                wih1 = [gw(9 + i) for i in range(3)]
                whh1 = [gw(12 + i) for i in range(3)]

                h0 = cpool.tile([P, NSH], BF, tag="h_L0")
                nc.gpsimd.memset(h0[:], 0.0)
                h1 = cpool.tile([P, NSH], BF, tag="h_L1")
                nc.gpsimd.memset(h1[:], 0.0)

                def gates_to_h(ps, gin_n, h, nm):
                    """ps cols [0:2N] hold r|z pre-activations (summed),
                    [2N:3N] gh_n; gin_n = gi n-gate AP. Updates h."""
                    rz = pool.tile([P, 2 * NSH], BF, tag=f"rz{nm}")
                    nc.scalar.activation(rz[:], ps[:, 0:2 * NSH], AF.Sigmoid)
                    ns_ = pool.tile([P, NSH], BF, tag=f"ns{nm}")
                    anye.tensor_mul(out=ns_[:], in0=rz[:, 0:NSH],
                                    in1=ps[:, 2 * NSH:3 * NSH])
                    anye.tensor_add(out=ns_[:], in0=ns_[:], in1=gin_n)
                    nc.scalar.activation(ns_[:], ns_[:], AF.Tanh)
                    hmn = pool.tile([P, NSH], BF, tag=f"hmn{nm}")
                    anye.tensor_sub(out=hmn[:], in0=h[:], in1=ns_[:])
                    anye.tensor_mul(out=hmn[:], in0=hmn[:],
                                    in1=rz[:, NSH:2 * NSH])
                    anye.tensor_add(out=h[:], in0=ns_[:], in1=hmn[:])

                for t in range(TR):
                    # load + transpose round-t embeddings: xt [feat-half, k, shot]
                    et = pool.tile([P, F2], BF, tag="et")
                    nc.sync.dma_start(out=et[:], in_=emb_d[t * P:(t + 1) * P, :])
                    xt = pool.tile([P, 2, P], BF, tag="xt")
                    for half in range(2):
                        tp = psA.tile([P, P], FP, tag="pA")
                        nc.tensor.matmul(tp[:], lhsT=et[:, half * P:(half + 1) * P],
                                         rhs=ident[:], start=True, stop=True)
                        anye.tensor_copy(out=xt[:, half, :], in_=tp[:])

                    # L0 step t: psum regions [r|z] = wih0@x_t + whh0@h0,
                    # [2N:3N] = whh0_n@h0, [3N:4N] = wih0_n@x_t
                    ps0 = psC.tile([P, 512], FP, tag="pC")
                    for gate in range(2):
                        reg = ps0[:, gate * NSH:(gate + 1) * NSH]
                        nc.tensor.matmul(reg, lhsT=whh0[gate], rhs=h0[:],
                                         start=True, stop=False)
                        nc.tensor.matmul(reg, lhsT=wih0[gate * 2], rhs=xt[:, 0, :],
                                         start=False, stop=False)
                        nc.tensor.matmul(reg, lhsT=wih0[gate * 2 + 1],
                                         rhs=xt[:, 1, :], start=False, stop=True)
                    nc.tensor.matmul(ps0[:, 2 * NSH:3 * NSH], lhsT=whh0[2],
                                     rhs=h0[:], start=True, stop=True)
                    nc.tensor.matmul(ps0[:, 3 * NSH:4 * NSH], lhsT=wih0[4],
                                     rhs=xt[:, 0, :], start=True, stop=False)
                    nc.tensor.matmul(ps0[:, 3 * NSH:4 * NSH], lhsT=wih0[5],
                                     rhs=xt[:, 1, :], start=False, stop=True)
                    gates_to_h(ps0, ps0[:, 3 * NSH:4 * NSH], h0, "0")

                    # L1 step t
                    ps1 = psC.tile([P, 512], FP, tag="pC")
                    for gate in range(2):
                        reg = ps1[:, gate * NSH:(gate + 1) * NSH]
                        nc.tensor.matmul(reg, lhsT=wih1[gate], rhs=h0[:],
                                         start=True, stop=False)
                        nc.tensor.matmul(reg, lhsT=whh1[gate], rhs=h1[:],
                                         start=False, stop=True)
                    nc.tensor.matmul(ps1[:, 2 * NSH:3 * NSH], lhsT=whh1[2],
                                     rhs=h1[:], start=True, stop=True)
                    nc.tensor.matmul(ps1[:, 3 * NSH:4 * NSH], lhsT=wih1[2],
                                     rhs=h0[:], start=True, stop=True)
                    gates_to_h(ps1, ps1[:, 3 * NSH:4 * NSH], h1, "1")

                hlast = h1
                lp = psA.tile([P, P], FP, tag="pA")
                nc.tensor.matmul(lp[:12, :NSH], lhsT=dec[:], rhs=hlast[:],
                                 start=True, stop=True)
                lo = pool.tile([12, NSH], FP, tag="lo")
                nc.vector.tensor_mul(out=lo[:], in0=lp[:12, :NSH], in1=am[:])
                nc.sync.dma_start(out=out_d[:], in_=lo[:])

            else:
                lo = pool.tile([12, NSH], FP, tag="lo")
                nc.gpsimd.memset(lo[:], 0.0)
                nc.sync.dma_start(out=out_d[:], in_=lo[:])

    nc.compile()
    return nc


def make_in_map(c, meta, W):
    """Per-core input arrays for run_bass_kernel_spmd."""
    A = c["arrays"]
    bf = lambda a: np.ascontiguousarray(a, dtype=BF16)
    f32 = lambda a: np.ascontiguousarray(a, dtype=np.float32)

    # w1full rows: slot*5+f -> wrel[f]; 80+f -> wroot[f]
    w1 = np.zeros((P, P), np.float32)
    w1[:F_IN] = f32(W["c1_wroot"])
    w1[F_IN:XROWS] = np.tile(f32(W["c1_wrel"]), (KSLOT, 1))
    wih0 = [f32(W["w_ih0"])[g * P:(g + 1) * P, k * P:(k + 1) * P].T
            for g in range(3) for k in range(2)]
    whh0 = [f32(W["w_hh0"])[g * P:(g + 1) * P, :].T for g in range(3)]
    wih1 = [f32(W["w_ih1"])[g * P:(g + 1) * P, :].T for g in range(3)]
    whh1 = [f32(W["w_hh1"])[g * P:(g + 1) * P, :].T for g in range(3)]
    gruw = np.stack(wih0 + whh0 + wih1 + whh1)
    amask = np.broadcast_to(A["amask"][None, :], (12, meta["n_shot_core"]))

    return {
        "xeV": bf(A["xeV"]),
        "xeSV": bf(A["xeSV"]),
        "xeH": bf(A["xeH"]),
        "gat": np.ascontiguousarray(wrap_idx16(A["gat_idx"], meta["E2_slots"])),
        "s2": bf(A["s2f"]),
        "s3": bf(A["s3"]),
        "pgid": np.ascontiguousarray(
            A["pool_gid"].reshape(-1, P, 1), dtype=np.int32),
        "amask": f32(amask),
        "ident": bf(np.eye(P, dtype=np.float32)),
        "w1": bf(w1),
        "wrel2": bf(W["c2_wrel"]),
        "wroot2": bf(W["c2_wroot"]),
        "gruw": bf(gruw),
        "dec": bf(W["dec_w"]),
    }


# ------------------------------------------------------------------
_CACHE = {}


def _get_nc(meta):
    key = (meta["NT_V"], meta["NT_H"], meta["n_SV_pad"], meta["E2_slots"],
           meta["G_rows"], tuple(meta["T_sub"]),
           tuple(tuple(r) for r in meta["wsub"]),
           tuple(meta["XR_V"]), tuple(meta["XR_SV"]), tuple(meta["XR_H"]))
    if key not in _CACHE:
        _CACHE[key] = build(meta, num_devices=NC)
    return _CACHE[key]


def kernel(**inputs):
    import sys as _sys
    if "/opt/trn_rl_repo" not in _sys.path:
        _sys.path.insert(0, "/opt/trn_rl_repo")
    from concourse.bass_utils import run_bass_kernel_spmd

    for k in ("c1_b", "c2_b", "b_ih0", "b_hh0", "b_ih1", "b_hh1", "dec_b",
              "empty_emb"):
        assert not np.any(np.asarray(inputs[k])), f"nonzero {k} unsupported"

    cores, meta = prep(inputs)
    W = {k: np.asarray(v, np.float32) for k, v in inputs.items()
         if k not in ("x", "edge_index", "edge_attr", "batch_labels",
                      "label_map", "B")}
    nc = _get_nc(meta)
    in_maps = [make_in_map(c, meta, W) for c in cores]
    res = None
    for attempt in range(4):
        try:
            res = run_bass_kernel_spmd(nc, in_maps, core_ids=list(range(NC)))
            break
        except Exception:
            if attempt == 3:
                raise
    B = meta["B"]
    out = np.zeros((B, 12), np.float32)
    nsh = meta["n_shot_core"]
    for d in range(NC):
        lg = res.results[d]["out"]          # [12, nsh]
        s = d + NC * np.arange(nsh)
        out[s[s < B]] = lg.T[s < B]
    return out

